# revision 1
# baseline (speedup 1.0000x reference)
"""Trainium2 Bass kernel for nn_APSDG (3-space GNN message passing).

8-core SPMD, dst-node sharding. Per layer:
  - own-shard node transform (logmap / l2norm pointwise + 128x128 matmuls)
    -> X_local [SH, 384] node-major
  - AllGather -> X_full [50176, 384] on every core
  - per 128-dst-node window: dma_gather X_full rows for the window's edges,
    one-hot matmul accumulation in PSUM (segment mean), pointwise post ops
    (LeakyReLU / expmap / l2norm) -> new embeddings.
Host side does integer-only edge prep (partitioning, padding, degree counts)
and layout; all float math runs on the NeuronCores.
"""
import sys

sys.path.insert(0, "/opt/trn_rl_repo")

import numpy as np

import concourse.bacc as bacc
import concourse.tile as tile
import concourse.mybir as mybir
from concourse.masks import make_identity

P = 128
F32 = mybir.dt.float32
I16 = mybir.dt.int16
EPS_LOG = 1e-10
EPS_L2 = 1e-12

# ---------------------------------------------------------------------------
# Workaround: this container's walrus codegen accepts only ONE sync-wait
# command per instruction, but Tile attaches several. Split the excess onto
# InstNoOps inserted before the instruction on the same engine (same-engine
# program order makes this equivalent for monotone sem-ge waits).
_ctr = [0]


def _split_excess_waits(nc, max_waits=1):
    def fresh():
        _ctr[0] += 1
        return f"WSPLIT-{_ctr[0]}"

    for f in nc.m.functions:
        for bb in f.blocks:
            insts = bb.instructions
            if not any(
                i.sync_info is not None and len(i.sync_info.on_wait) > max_waits
                for i in insts
            ):
                continue
            out = []
            for inst in insts:
                si = inst.sync_info
                if si is not None and len(si.on_wait) > max_waits:
                    waits = list(si.on_wait)
                    ge = [w for w in waits if "ge" in (w.wait_mode or "")]
                    eq = [w for w in waits if w not in ge]
                    keep = (eq + ge)[-max_waits:] if not eq else eq[-max_waits:]
                    hoist = [w for w in waits if w not in keep]
                    if len(keep) > max_waits:
                        raise RuntimeError(
                            f"{inst.name}: cannot split {len(eq)} eq-mode waits"
                        )
                    for i in range(0, len(hoist), max_waits):
                        nop = mybir.InstNoOp(name=fresh(), ins=[], outs=[])
                        nop.engine = inst.engine
                        nop.sync_info = mybir.SyncInfo(
                            on_wait=hoist[i : i + max_waits], on_update=[]
                        )
                        out.append(nop)
                    si.on_wait = keep
                out.append(inst)
            bb.instructions = out


# ---------------------------------------------------------------------------

P = 128
F32 = mybir.dt.float32
BF16 = mybir.dt.bfloat16
XDT = F32  # storage dtype for gathered X; BF16 fails the fp32-envelope gate
I16 = mybir.dt.int16
EPS_LOG = 1e-10   # log/exp map norm clamp (reference EPS)
EPS_L2 = 1e-12    # l2norm clamp


class _Cfg:
    def __init__(self, n_nodes, n_edges, n_cores=8, w_per_core=49, chunk=7,
                 base=32768, d=128, n_layers=2, split_chunks=None):
        self.N = n_nodes
        self.E = n_edges
        self.NC = n_cores
        self.W = w_per_core            # windows (128 dst nodes) per core
        self.CH = chunk                # node-tiles per processing chunk
        assert w_per_core % chunk == 0
        self.NG = w_per_core // chunk  # chunks/groups per core
        self.SH = w_per_core * P       # shard rows per core
        self.NPAD = self.SH * n_cores
        assert self.NPAD >= n_nodes
        self.BASE = base
        if split_chunks is None:
            split_chunks = max(1, (self.NG * 3) // 7)
        self.SPC = split_chunks            # chunks in the A half
        self.RA = split_chunks * chunk * P  # rows per core in half A
        self.RB = self.SH - self.RA
        assert n_cores * self.RA <= 32768 and n_cores * self.RB <= 32768
        self.D = d
        self.DX = 3 * d
        self.L = n_layers


def _host_prep(cfg, src, dst):
    """Integer-only edge prep. Returns per-core arrays + static meta."""
    NC, W, SH = cfg.NC, cfg.W, cfg.SH
    RA, RB = cfg.RA, cfg.RB
    src = np.asarray(src, np.int64)
    dst = np.asarray(dst, np.int64)

    core = dst // SH
    local = dst - core * SH
    win = local // P
    slot = local % P
    src_core = src // SH
    src_r = src - src_core * SH
    is_hi = src_r >= RA
    src_remap = np.where(is_hi, src_core * RB + (src_r - RA),
                         src_core * RA + src_r)

    # group edges by (core, window, is_hi): order by key, stable
    key = (core * W + win) * 2 + is_hi
    order = np.argsort(key, kind="stable")
    key_s = key[order]
    src_s = src_remap[order]
    slot_s = slot[order]
    # counts per (c, w, half)
    cnt = np.bincount(key_s, minlength=NC * W * 2).reshape(NC, W, 2)
    starts = np.zeros(NC * W * 2 + 1, np.int64)
    np.cumsum(cnt.reshape(-1), out=starts[1:])

    V_lo = cnt[:, :, 0]
    V_hi = cnt[:, :, 1]
    K_lo = ((V_lo.max(axis=0) + P - 1) // P) * P     # [W] uniform across cores
    K_hi = ((V_hi.max(axis=0) + P - 1) // P) * P
    T_lo = K_lo // P
    T_hi = K_hi // P
    T = T_lo + T_hi
    tile_base = np.zeros(W + 1, np.int64)
    np.cumsum(T, out=tile_base[1:])
    TT = int(tile_base[-1])                           # total tiles per core
    IC = TT * P // 16                                 # idx cols (int16 wrap)

    GMAX = 1024  # must match build_nc's per-gather split
    idx_all = np.zeros((NC, P, IC), np.int16)
    dstv = np.full((NC, P, TT), -1.0, np.float32)
    gcnt = [[] for _ in range(NC)]  # per-core valid count per sub-gather
    for c in range(NC):
        for w in range(W):
            for half in (0, 1):
                K = int(K_lo[w] if half == 0 else K_hi[w])
                if K == 0:
                    continue
                s0 = starts[(c * W + w) * 2 + half]
                s1 = starts[(c * W + w) * 2 + half + 1]
                e_src = src_s[s0:s1]
                e_slot = slot_s[s0:s1]
                V = len(e_src)
                idx_pad = np.full(K, -1, np.int64)
                idx_pad[:V] = e_src
                sl_pad = np.full(K, -1.0, np.float32)
                sl_pad[: len(e_slot)] = e_slot
                # per sub-gather: valid count; force >=1 valid (dummy idx 0,
                # its dstv slot stays -1 so it contributes nothing)
                for off in range(0, K, GMAX):
                    kk = min(GMAX, K - off)
                    v_here = min(max(V - off, 0), kk)
                    if v_here == 0:
                        idx_pad[off] = 0
                        v_here = 1
                    gcnt[c].append(v_here)
                tb = int(tile_base[w] + (0 if half == 0 else T_lo[w]))
                # idx wrap: index j -> [j%16, col_base + j//16], replicated x8
                wrap = idx_pad.reshape(-1, 16).T.astype(np.int16)  # [16, K/16]
                cb = tb * P // 16
                idx_all[c, :, cb : cb + K // 16] = np.tile(wrap, (8, 1))
                dstv[c, :, tb : tb + K // P] = sl_pad.reshape(-1, P).T
    gcnt = np.asarray(gcnt, np.int32)[:, None, :]  # [NC, 1, NGATH]

    deg = np.bincount(dst, minlength=cfg.NPAD).astype(np.float32)
    deg_arr = deg.reshape(NC, W, P).transpose(0, 2, 1).copy()  # [NC, 128, W]

    meta = dict(K_lo=K_lo.tolist(), K_hi=K_hi.tolist(),
                T_lo=T_lo.tolist(), T_hi=T_hi.tolist(), T=T.tolist(),
                tile_base=tile_base.tolist(), TT=TT, IC=IC,
                NGATH=int(gcnt.shape[2]))
    return idx_all, dstv, deg_arr, gcnt, meta


def _build_nc(cfg, meta):
    NC_, W, CH, NG, SH, NPAD, D, DX, L = (
        cfg.NC, cfg.W, cfg.CH, cfg.NG, cfg.SH, cfg.NPAD,
        cfg.D, cfg.DX, cfg.L)
    RA, RB, SPC = cfg.RA, cfg.RB, cfg.SPC
    TT, IC = meta["TT"], meta["IC"]
    K_lo, K_hi = meta["K_lo"], meta["K_hi"]
    T_lo, T_hi, T, tile_base = meta["T_lo"], meta["T_hi"], meta["T"], meta["tile_base"]
    Tmax = max(T)

    nc = bacc.Bacc("TRN2", target_bir_lowering=False, debug=False,
                   num_devices=NC_)

    emb0_d = nc.declare_dram_parameter("emb0", [SH, DX], F32, isOutput=False)
    wT_d = nc.declare_dram_parameter("wT", [L, 3, D, D], F32, isOutput=False)
    bias_d = nc.declare_dram_parameter("bias", [L, 3, D], F32, isOutput=False)
    idx_d = nc.declare_dram_parameter("idx", [P, IC], I16, isOutput=False)
    dstv_d = nc.declare_dram_parameter("dstv", [P, TT], F32, isOutput=False)
    deg_d = nc.declare_dram_parameter("deg", [P, W], F32, isOutput=False)
    iota_d = nc.declare_dram_parameter("iota", [P, P], F32, isOutput=False)
    curv_d = nc.declare_dram_parameter("curv", [P, 1], F32, isOutput=False)
    gcnt_d = nc.declare_dram_parameter("gcnt", [1, meta["NGATH"]],
                                       mybir.dt.int32, isOutput=False)
    out_d = nc.declare_dram_parameter("out", [SH, DX], F32, isOutput=True)

    emb_mid = [nc.dram_tensor(f"emb_mid{g}", [CH * P, DX], F32)
               for g in range(NG)]
    x_locA = [nc.dram_tensor(f"x_locA{l}", [RA, DX], XDT) for l in range(L)]
    x_locB = [nc.dram_tensor(f"x_locB{l}", [RB, DX], XDT) for l in range(L)]
    xfA = [nc.dram_tensor(f"xfA{l}", [NC_ * RA, DX], XDT, addr_space="Shared")
           for l in range(L)]
    xfB = [nc.dram_tensor(f"xfB{l}", [NC_ * RB, DX], XDT, addr_space="Shared")
           for l in range(L)]

    def rows3d(dram_ap, r0, ntiles):
        """DRAM rows [r0, r0+ntiles*128) as [128, ntiles, DX]."""
        return dram_ap[r0 : r0 + ntiles * P, :].rearrange(
            "(j p) d -> p j d", p=P)

    from contextlib import ExitStack
    with tile.TileContext(nc) as tc, ExitStack() as es:
        cpool = es.enter_context(tc.tile_pool(name="const", bufs=1))
        spool = es.enter_context(tc.tile_pool(name="work", bufs=2))
        gpool = es.enter_context(tc.tile_pool(name="gath", bufs=3))
        rpool = es.enter_context(tc.tile_pool(name="onehot", bufs=4))
        ppool = es.enter_context(tc.tile_pool(name="psum", bufs=2, space="PSUM"))
        ppool2 = es.enter_context(tc.tile_pool(name="psum2", bufs=2, space="PSUM"))

        # ---- constants ----
        iota_t = cpool.tile([P, P], F32)
        nc.sync.dma_start(out=iota_t[:], in_=iota_d[:])
        ident_t = cpool.tile([P, P], F32)
        make_identity(nc, ident_t[:])
        idx_t = cpool.tile([P, IC], I16)
        nc.sync.dma_start(out=idx_t[:], in_=idx_d[:])
        dstv_t = cpool.tile([P, TT], F32)
        nc.sync.dma_start(out=dstv_t[:], in_=dstv_d[:])
        deg_t = cpool.tile([P, W], F32)
        nc.sync.dma_start(out=deg_t[:], in_=deg_d[:])
        gcnt_t = cpool.tile([1, meta["NGATH"]], mybir.dt.int32)
        nc.sync.dma_start(out=gcnt_t[:], in_=gcnt_d[:])
        # zero the gather-buffer slots once: slots skipped by runtime-count
        # gathers keep stale data, and the one-hot matmul would turn virgin
        # (NaN) SBUF into 0*NaN=NaN despite the zero one-hot column.
        for _wi in range(3):
            wt = gpool.tile([P, Tmax, DX], XDT, tag="gb", name="gb")
            nc.vector.memset(wt[:], 0.0)
        ones1_t = cpool.tile([1, P], F32)
        nc.vector.memset(ones1_t[:], 1.0)
        wT_t = [[cpool.tile([D, D], F32, name=f"wT{l}{s}", tag=f"wT{l}{s}") for s in range(3)]
                for l in range(L)]
        bias_t = [[cpool.tile([1, D], F32, name=f"bias{l}{s}", tag=f"bias{l}{s}") for s in range(3)]
                  for l in range(L)]
        for l in range(L):
            for s in range(3):
                nc.sync.dma_start(out=wT_t[l][s][:], in_=wT_d[l, s])
                nc.sync.dma_start(out=bias_t[l][s][:], in_=bias_d[l, s][None, :])

        # recip = 1/max(deg,1)
        recip_t = cpool.tile([P, W], F32)
        nc.vector.tensor_scalar_max(out=recip_t[:], in0=deg_t[:], scalar1=1.0)
        nc.vector.reciprocal(out=recip_t[:], in_=recip_t[:])

        # curvature-derived scalars [128,1]
        curv_t = cpool.tile([P, 1], F32)
        nc.sync.dma_start(out=curv_t[:], in_=curv_d[:])
        sc_t = cpool.tile([P, 1], F32)       # sqrt(c)
        inv_sc_t = cpool.tile([P, 1], F32)   # 1/sqrt(c)
        nc.scalar.activation(out=sc_t[:], in_=curv_t[:],
                             func=mybir.ActivationFunctionType.Sqrt)
        nc.vector.reciprocal(out=inv_sc_t[:], in_=sc_t[:])

        def l2norm_chunk(xap, ntiles, eps, pfx=""):
            """In-place row-l2norm of [128, ntiles, 128] slice."""
            sq = spool.tile([P, ntiles, D], F32, tag=pfx + "sq", name="sq")
            nc.vector.tensor_tensor(out=sq[:], in0=xap, in1=xap,
                                    op=mybir.AluOpType.mult)
            n_ = spool.tile([P, ntiles], F32, tag=pfx + "nrm", name="nrm")
            nc.vector.tensor_reduce(out=n_[:], in_=sq[:],
                                    axis=mybir.AxisListType.X,
                                    op=mybir.AluOpType.add)
            nc.scalar.activation(out=n_[:], in_=n_[:],
                                 func=mybir.ActivationFunctionType.Sqrt)
            nc.vector.tensor_scalar_max(out=n_[:], in0=n_[:], scalar1=eps)
            nc.vector.reciprocal(out=n_[:], in_=n_[:])
            for j in range(ntiles):
                nc.vector.tensor_scalar_mul(out=xap[:, j, :], in0=xap[:, j, :],
                                            scalar1=n_[:, j : j + 1])

        def emit_build_chunk(l, g):
            r0 = g * CH * P
            ch = spool.tile([P, CH, DX], F32, tag="embch", name="embch")
            if l == 0:
                nc.sync.dma_start(out=ch[:], in_=rows3d(emb0_d[:], r0, CH))
            else:
                nc.sync.dma_start(out=ch[:], in_=rows3d(emb_mid[g][:], 0, CH))

            # tangent = logmap(b_emb): b cols [D, 2D)
            bpart = ch[:, :, D : 2 * D]
            sq = spool.tile([P, CH, D], F32, tag="sq", name="sq")
            nc.vector.tensor_tensor(out=sq[:], in0=bpart, in1=bpart,
                                    op=mybir.AluOpType.mult)
            n_ = spool.tile([P, CH], F32, tag="nrm", name="nrm")
            nc.vector.tensor_reduce(out=n_[:], in_=sq[:],
                                    axis=mybir.AxisListType.X,
                                    op=mybir.AluOpType.add)
            nc.scalar.activation(out=n_[:], in_=n_[:],
                                 func=mybir.ActivationFunctionType.Sqrt)
            nc.vector.tensor_scalar_max(out=n_[:], in0=n_[:], scalar1=EPS_LOG)
            a_ = spool.tile([P, CH], F32, tag="a_", name="a_")
            nc.vector.tensor_scalar(out=a_[:], in0=n_[:],
                                    scalar1=sc_t[:, 0:1], scalar2=None,
                                    op0=mybir.AluOpType.mult)
            t1 = spool.tile([P, CH], F32, tag="t1", name="t1")
            nc.scalar.activation(out=t1[:], in_=a_[:],
                                 func=mybir.ActivationFunctionType.Ln,
                                 bias=1.0, scale=1.0)
            t2 = spool.tile([P, CH], F32, tag="t2", name="t2")
            nc.scalar.activation(out=t2[:], in_=a_[:],
                                 func=mybir.ActivationFunctionType.Ln,
                                 bias=1.0, scale=-1.0)
            f_ = spool.tile([P, CH], F32, tag="f_", name="f_")
            nc.vector.tensor_tensor(out=f_[:], in0=t1[:], in1=t2[:],
                                    op=mybir.AluOpType.subtract)
            rn = spool.tile([P, CH], F32, tag="rn", name="rn")
            nc.vector.reciprocal(out=rn[:], in_=n_[:])
            nc.vector.tensor_tensor(out=f_[:], in0=f_[:], in1=rn[:],
                                    op=mybir.AluOpType.mult)
            nc.vector.tensor_scalar(out=f_[:], in0=f_[:],
                                    scalar1=inv_sc_t[:, 0:1], scalar2=None,
                                    op0=mybir.AluOpType.mult)
            tan = spool.tile([P, CH, D], F32, tag="tan", name="tan")
            for j in range(CH):
                nc.vector.tensor_scalar_mul(out=tan[:, j, :],
                                            in0=bpart[:, j, :],
                                            scalar1=f_[:, j : j + 1])

            sn = spool.tile([P, CH, D], F32, tag="sn", name="sn")
            nc.vector.tensor_copy(out=sn[:], in_=ch[:, :, 2 * D : 3 * D])
            l2norm_chunk(sn[:], CH, EPS_L2)

            xch = spool.tile([P, CH, DX], XDT, tag="xch", name="xch")
            for j in range(CH):
                ins_nm = (ch[:, j, 0:D], tan[:, j, :], sn[:, j, :])
                for sp_ in range(3):
                    tp = ppool2.tile([P, P], F32, space="PSUM", tag="tp",
                                     name="tp")
                    nc.tensor.transpose(out=tp[:], in_=ins_nm[sp_],
                                        identity=ident_t[:])
                    tsb = spool.tile([P, P], F32, tag="tsb", name="tsb")
                    nc.any.tensor_copy(out=tsb[:], in_=tp[:])
                    xp = ppool2.tile([P, D], F32, space="PSUM", tag="xp",
                                     name="xp")
                    nc.tensor.matmul(xp[:], lhsT=tsb[:], rhs=wT_t[l][sp_][:],
                                     start=True, stop=False)
                    nc.tensor.matmul(xp[:], lhsT=ones1_t[:],
                                     rhs=bias_t[l][sp_][:],
                                     start=False, stop=True)
                    nc.any.tensor_copy(out=xch[:, j, sp_ * D : (sp_ + 1) * D],
                                       in_=xp[:])
            l2norm_chunk(xch[:, :, 2 * D : 3 * D], CH, EPS_L2)
            if g < SPC:
                nc.sync.dma_start(out=rows3d(x_locA[l][:], r0, CH), in_=xch[:])
            else:
                nc.sync.dma_start(
                    out=rows3d(x_locB[l][:], r0 - RA, CH), in_=xch[:])

        def emit_allgather_a(l):
            nc.gpsimd.collective_compute(
                "AllGather", mybir.AluOpType.bypass,
                replica_groups=[list(range(NC_))],
                ins=[x_locA[l][:]], outs=[xfA[l][:]])

        def emit_allgather_b(l):
            nc.gpsimd.collective_compute(
                "AllGather", mybir.AluOpType.bypass,
                replica_groups=[list(range(NC_))],
                ins=[x_locB[l][:]], outs=[xfB[l][:]])

        gcnt_regs = [nc.gpsimd.alloc_register(f"gcnt_reg{i}")
                     for i in range(4)]
        _gri = [0]

        def gather_ordinal(w, half, off):
            # position of this sub-gather in host_prep's gcnt emission order
            gi = 0
            for w2 in range(w):
                gi += (K_lo[w2] + 1023) // 1024 + (K_hi[w2] + 1023) // 1024
            if half == 1:
                gi += (K_lo[w] + 1023) // 1024
            return gi + off // 1024

        def emit_agg_group(l, grp):
            pb = spool.tile([P, CH, DX], F32, tag="postbuf", name="postbuf")
            for wi in range(CH):
                w = grp * CH + wi
                tb = tile_base[w]
                gb = gpool.tile([P, Tmax, DX], XDT, tag="gb", name="gb")
                GMAX = 1024  # max idxs per dma_gather (descriptor ring cap)
                for half, K, t0_ in ((0, K_lo[w], 0), (1, K_hi[w], T_lo[w])):
                    src_ap = xfA[l][:] if half == 0 else xfB[l][:]
                    for off in range(0, K, GMAX):
                        kk = min(GMAX, K - off)
                        tstart = t0_ + off // P
                        cb = (tb + tstart) * P // 16
                        gi = gather_ordinal(w, half, off)
                        rv = gcnt_regs[_gri[0] % 4]
                        _gri[0] += 1
                        nc.gpsimd.reg_load(rv, gcnt_t[0:1, gi : gi + 1])
                        nc.gpsimd.dma_gather(
                            out_ap=gb[:, tstart : tstart + kk // P, :],
                            in_ap=src_ap,
                            idxs_ap=idx_t[:, cb : cb + kk // 16],
                            num_idxs=kk, num_idxs_reg=rv,
                            elem_size=DX)
                acc = ppool.tile([P, DX], F32, space="PSUM", tag="acc",
                                 name="acc")
                for t in range(T[w]):
                    r_ = rpool.tile([P, P], XDT, tag="r", name="r")
                    nc.vector.tensor_tensor(
                        out=r_[:],
                        in0=dstv_t[:, tb + t : tb + t + 1].to_broadcast([P, P]),
                        in1=iota_t[:],
                        op=mybir.AluOpType.is_equal)
                    nc.tensor.matmul(acc[:], lhsT=r_[:], rhs=gb[:, t, :],
                                     start=(t == 0), stop=(t == T[w] - 1))
                nc.vector.tensor_scalar_mul(out=pb[:, wi, :], in0=acc[:],
                                            scalar1=recip_t[:, w : w + 1])

            # post pointwise
            epart = pb[:, :, 0:D]
            tmp = spool.tile([P, CH, D], F32, tag="psq", name="psq")
            nc.vector.tensor_scalar_mul(out=tmp[:], in0=epart, scalar1=0.2)
            nc.vector.tensor_tensor(out=epart, in0=epart, in1=tmp[:],
                                    op=mybir.AluOpType.max)
            bpart = pb[:, :, D : 2 * D]
            nc.vector.tensor_tensor(out=tmp[:], in0=bpart, in1=bpart,
                                    op=mybir.AluOpType.mult)
            n_ = spool.tile([P, CH], F32, tag="pnrm", name="pnrm")
            nc.vector.tensor_reduce(out=n_[:], in_=tmp[:],
                                    axis=mybir.AxisListType.X,
                                    op=mybir.AluOpType.add)
            nc.scalar.activation(out=n_[:], in_=n_[:],
                                 func=mybir.ActivationFunctionType.Sqrt)
            nc.vector.tensor_scalar_max(out=n_[:], in0=n_[:], scalar1=EPS_LOG)
            a_ = spool.tile([P, CH], F32, tag="pa_", name="pa_")
            nc.vector.tensor_scalar(out=a_[:], in0=n_[:],
                                    scalar1=sc_t[:, 0:1], scalar2=None,
                                    op0=mybir.AluOpType.mult)
            th = spool.tile([P, CH], F32, tag="pt1", name="pt1")
            nc.scalar.activation(out=th[:], in_=a_[:],
                                 func=mybir.ActivationFunctionType.Tanh,
                                 scale=0.5)
            ra = spool.tile([P, CH], F32, tag="pt2", name="pt2")
            nc.vector.reciprocal(out=ra[:], in_=a_[:])
            nc.vector.tensor_tensor(out=th[:], in0=th[:], in1=ra[:],
                                    op=mybir.AluOpType.mult)
            for j in range(CH):
                nc.vector.tensor_scalar_mul(out=bpart[:, j, :],
                                            in0=bpart[:, j, :],
                                            scalar1=th[:, j : j + 1])
            l2norm_chunk(pb[:, :, 2 * D : 3 * D], CH, EPS_L2, pfx="p")

            if l == L - 1:
                nc.sync.dma_start(
                    out=rows3d(out_d[:], grp * CH * P, CH), in_=pb[:])
            else:
                nc.sync.dma_start(
                    out=rows3d(emb_mid[grp][:], 0, CH), in_=pb[:])

        # driver: interleave layer-(l+1) build chunks into layer-l agg
        # groups; AllGather half A fires once its chunks are built (half
        # tensors keep the WAR deps exact).
        for g in range(NG):
            emit_build_chunk(0, g)
            if g == SPC - 1:
                emit_allgather_a(0)
        emit_allgather_b(0)
        for g in range(NG):
            emit_agg_group(0, g)
            emit_build_chunk(1, g)
            if g == SPC - 1:
                emit_allgather_a(1)
        emit_allgather_b(1)
        for g in range(NG):
            emit_agg_group(1, g)

    return nc


def _build_in_maps(cfg, src, dst, e_emb, b_emb, s_emb, e_W, e_b, b_W, b_b,
                   s_W, s_b, b_curvature):
    N, SH, NC, DX, L, D = cfg.N, cfg.SH, cfg.NC, cfg.DX, cfg.L, cfg.D
    idx_all, dstv, deg_arr, gcnt, meta = _host_prep(cfg, src, dst)

    emb_full = np.zeros((cfg.NPAD, DX), np.float32)
    emb_full[:N, 0:D] = e_emb
    emb_full[:N, D:2 * D] = b_emb
    emb_full[:N, 2 * D:3 * D] = s_emb

    wT = np.stack([
        np.stack([e_W[l].T, b_W[l].T, s_W[l].T]) for l in range(L)
    ]).astype(np.float32)
    bias = np.stack([
        np.stack([e_b[l], b_b[l], s_b[l]]) for l in range(L)
    ]).astype(np.float32)

    iota = np.tile(np.arange(P, dtype=np.float32), (P, 1))
    curv = np.full((P, 1), np.float32(np.asarray(b_curvature).reshape(-1)[0]))

    in_maps = []
    for c in range(NC):
        in_maps.append({
            "emb0": np.ascontiguousarray(emb_full[c * SH:(c + 1) * SH]),
            "wT": wT,
            "bias": bias,
            "idx": np.ascontiguousarray(idx_all[c]),
            "dstv": np.ascontiguousarray(dstv[c]),
            "deg": np.ascontiguousarray(deg_arr[c]),
            "iota": iota,
            "curv": curv,
            "gcnt": np.ascontiguousarray(gcnt[c]),
        })
    return in_maps, meta


_LAST = {}


def run_kernel(inputs, trace=False):
    """Full pipeline; returns (results, exec_time_ns)."""
    from concourse.bass_utils import run_bass_kernel_spmd

    cfg = _Cfg(n_nodes=50000, n_edges=800000)
    src = np.asarray(inputs["src"], np.int32)
    dst = np.asarray(inputs["dst"], np.int32)
    in_maps, meta = _build_in_maps(
        cfg, src, dst,
        np.asarray(inputs["e_emb"], np.float32),
        np.asarray(inputs["b_emb"], np.float32),
        np.asarray(inputs["s_emb"], np.float32),
        np.asarray(inputs["e_W"], np.float32),
        np.asarray(inputs["e_b"], np.float32),
        np.asarray(inputs["b_W"], np.float32),
        np.asarray(inputs["b_b"], np.float32),
        np.asarray(inputs["s_W"], np.float32),
        np.asarray(inputs["s_b"], np.float32),
        np.asarray(inputs["b_curvature"], np.float32))

    # reuse the compiled program across calls when the graph layout matches
    gkey = (meta["TT"], hash(src.tobytes()) ^ hash(dst.tobytes()))
    nc = _LAST.get(gkey)
    if nc is None:
        nc = _build_nc(cfg, meta)
        nc.finalize()
        _split_excess_waits(nc)
        _LAST.clear()
        _LAST[gkey] = nc

    res = run_bass_kernel_spmd(nc, in_maps, core_ids=list(range(cfg.NC)),
                               trace=trace)
    full = np.concatenate([res.results[c]["out"] for c in range(cfg.NC)],
                          axis=0)[: cfg.N]
    D = cfg.D
    outs = (np.ascontiguousarray(full[:, 0:D]),
            np.ascontiguousarray(full[:, D:2 * D]),
            np.ascontiguousarray(full[:, 2 * D:3 * D]))
    return outs, res.exec_time_ns


def kernel(**inputs):
    outs, _ = run_kernel(inputs, trace=False)
    return outs



# revision 3
# speedup vs baseline: 1.0330x; 1.0330x over previous
"""Trainium2 Bass kernel for nn_APSDG (3-space GNN message passing).

8-core SPMD, dst-node sharding. Per layer:
  - own-shard node transform (logmap / l2norm pointwise + 128x128 matmuls)
    -> X_local [SH, 384] node-major
  - AllGather -> X_full [50176, 384] on every core
  - per 128-dst-node window: dma_gather X_full rows for the window's edges,
    one-hot matmul accumulation in PSUM (segment mean), pointwise post ops
    (LeakyReLU / expmap / l2norm) -> new embeddings.
Host side does integer-only edge prep (partitioning, padding, degree counts)
and layout; all float math runs on the NeuronCores.
"""
import sys

sys.path.insert(0, "/opt/trn_rl_repo")

import numpy as np

import concourse.bacc as bacc
import concourse.tile as tile
import concourse.mybir as mybir
from concourse.masks import make_identity

P = 128
F32 = mybir.dt.float32
I16 = mybir.dt.int16
EPS_LOG = 1e-10
EPS_L2 = 1e-12

# ---------------------------------------------------------------------------
# Workaround: this container's walrus codegen accepts only ONE sync-wait
# command per instruction, but Tile attaches several. Split the excess onto
# InstNoOps inserted before the instruction on the same engine (same-engine
# program order makes this equivalent for monotone sem-ge waits).
_ctr = [0]


def _split_excess_waits(nc, max_waits=1):
    def fresh():
        _ctr[0] += 1
        return f"WSPLIT-{_ctr[0]}"

    for f in nc.m.functions:
        for bb in f.blocks:
            insts = bb.instructions
            if not any(
                i.sync_info is not None and len(i.sync_info.on_wait) > max_waits
                for i in insts
            ):
                continue
            out = []
            for inst in insts:
                si = inst.sync_info
                if si is not None and len(si.on_wait) > max_waits:
                    waits = list(si.on_wait)
                    ge = [w for w in waits if "ge" in (w.wait_mode or "")]
                    eq = [w for w in waits if w not in ge]
                    keep = (eq + ge)[-max_waits:] if not eq else eq[-max_waits:]
                    hoist = [w for w in waits if w not in keep]
                    if len(keep) > max_waits:
                        raise RuntimeError(
                            f"{inst.name}: cannot split {len(eq)} eq-mode waits"
                        )
                    for i in range(0, len(hoist), max_waits):
                        nop = mybir.InstNoOp(name=fresh(), ins=[], outs=[])
                        nop.engine = inst.engine
                        nop.sync_info = mybir.SyncInfo(
                            on_wait=hoist[i : i + max_waits], on_update=[]
                        )
                        out.append(nop)
                    si.on_wait = keep
                out.append(inst)
            bb.instructions = out


# ---------------------------------------------------------------------------

P = 128
F32 = mybir.dt.float32
BF16 = mybir.dt.bfloat16
XDT = F32  # storage dtype for gathered X; BF16 fails the fp32-envelope gate
I16 = mybir.dt.int16
EPS_LOG = 1e-10   # log/exp map norm clamp (reference EPS)
EPS_L2 = 1e-12    # l2norm clamp


class _Cfg:
    def __init__(self, n_nodes, n_edges, n_cores=8, w_per_core=49, chunk=7,
                 base=32768, d=128, n_layers=2, split_chunks=None):
        self.N = n_nodes
        self.E = n_edges
        self.NC = n_cores
        self.W = w_per_core            # windows (128 dst nodes) per core
        self.CH = chunk                # node-tiles per processing chunk
        assert w_per_core % chunk == 0
        self.NG = w_per_core // chunk  # chunks/groups per core
        self.SH = w_per_core * P       # shard rows per core
        self.NPAD = self.SH * n_cores
        assert self.NPAD >= n_nodes
        self.BASE = base
        if split_chunks is None:
            split_chunks = max(1, (self.NG * 3) // 7)
        self.SPC = split_chunks            # chunks in the A half
        self.RA = split_chunks * chunk * P  # rows per core in half A
        self.RB = self.SH - self.RA
        assert n_cores * self.RA <= 32768 and n_cores * self.RB <= 32768
        self.D = d
        self.DX = 3 * d
        self.L = n_layers


def _host_prep(cfg, src, dst):
    """Integer-only edge prep. Returns per-core arrays + static meta."""
    NC, W, SH = cfg.NC, cfg.W, cfg.SH
    RA, RB = cfg.RA, cfg.RB
    src = np.asarray(src, np.int64)
    dst = np.asarray(dst, np.int64)

    core = dst // SH
    local = dst - core * SH
    win = local // P
    slot = local % P
    src_core = src // SH
    src_r = src - src_core * SH
    is_hi = src_r >= RA
    src_remap = np.where(is_hi, src_core * RB + (src_r - RA),
                         src_core * RA + src_r)

    # group edges by (core, window, is_hi): order by key, stable
    key = (core * W + win) * 2 + is_hi
    order = np.argsort(key, kind="stable")
    key_s = key[order]
    src_s = src_remap[order]
    slot_s = slot[order]
    # counts per (c, w, half)
    cnt = np.bincount(key_s, minlength=NC * W * 2).reshape(NC, W, 2)
    starts = np.zeros(NC * W * 2 + 1, np.int64)
    np.cumsum(cnt.reshape(-1), out=starts[1:])

    V_lo = cnt[:, :, 0]
    V_hi = cnt[:, :, 1]
    K_lo = ((V_lo.max(axis=0) + P - 1) // P) * P     # [W] uniform across cores
    K_hi = ((V_hi.max(axis=0) + P - 1) // P) * P
    T_lo = K_lo // P
    T_hi = K_hi // P
    T = T_lo + T_hi
    tile_base = np.zeros(W + 1, np.int64)
    np.cumsum(T, out=tile_base[1:])
    TT = int(tile_base[-1])                           # total tiles per core
    IC = TT * P // 16                                 # idx cols (int16 wrap)

    GMAX = 1024  # must match build_nc's per-gather split
    idx_all = np.zeros((NC, P, IC), np.int16)
    dstv = np.full((NC, P, TT), -1.0, np.float32)
    gcnt = [[] for _ in range(NC)]  # per-core valid count per sub-gather
    for c in range(NC):
        for w in range(W):
            for half in (0, 1):
                K = int(K_lo[w] if half == 0 else K_hi[w])
                if K == 0:
                    continue
                s0 = starts[(c * W + w) * 2 + half]
                s1 = starts[(c * W + w) * 2 + half + 1]
                e_src = src_s[s0:s1]
                e_slot = slot_s[s0:s1]
                V = len(e_src)
                idx_pad = np.full(K, -1, np.int64)
                idx_pad[:V] = e_src
                sl_pad = np.full(K, -1.0, np.float32)
                sl_pad[: len(e_slot)] = e_slot
                # per sub-gather: valid count; force >=1 valid (dummy idx 0,
                # its dstv slot stays -1 so it contributes nothing)
                for off in range(0, K, GMAX):
                    kk = min(GMAX, K - off)
                    v_here = min(max(V - off, 0), kk)
                    if v_here == 0:
                        idx_pad[off] = 0
                        v_here = 1
                    gcnt[c].append(v_here)
                tb = int(tile_base[w] + (0 if half == 0 else T_lo[w]))
                # idx wrap: index j -> [j%16, col_base + j//16], replicated x8
                wrap = idx_pad.reshape(-1, 16).T.astype(np.int16)  # [16, K/16]
                cb = tb * P // 16
                idx_all[c, :, cb : cb + K // 16] = np.tile(wrap, (8, 1))
                dstv[c, :, tb : tb + K // P] = sl_pad.reshape(-1, P).T
    gcnt = np.asarray(gcnt, np.int32)[:, None, :]  # [NC, 1, NGATH]

    deg = np.bincount(dst, minlength=cfg.NPAD).astype(np.float32)
    deg_arr = deg.reshape(NC, W, P).transpose(0, 2, 1).copy()  # [NC, 128, W]

    meta = dict(K_lo=K_lo.tolist(), K_hi=K_hi.tolist(),
                T_lo=T_lo.tolist(), T_hi=T_hi.tolist(), T=T.tolist(),
                tile_base=tile_base.tolist(), TT=TT, IC=IC,
                NGATH=int(gcnt.shape[2]))
    return idx_all, dstv, deg_arr, gcnt, meta


def _build_nc(cfg, meta):
    NC_, W, CH, NG, SH, NPAD, D, DX, L = (
        cfg.NC, cfg.W, cfg.CH, cfg.NG, cfg.SH, cfg.NPAD,
        cfg.D, cfg.DX, cfg.L)
    RA, RB, SPC = cfg.RA, cfg.RB, cfg.SPC
    TT, IC = meta["TT"], meta["IC"]
    K_lo, K_hi = meta["K_lo"], meta["K_hi"]
    T_lo, T_hi, T, tile_base = meta["T_lo"], meta["T_hi"], meta["T"], meta["tile_base"]
    Tmax = max(T)

    nc = bacc.Bacc("TRN2", target_bir_lowering=False, debug=False,
                   num_devices=NC_)

    emb0_d = nc.declare_dram_parameter("emb0", [SH, DX], F32, isOutput=False)
    wT_d = nc.declare_dram_parameter("wT", [L, 3, D, D], F32, isOutput=False)
    bias_d = nc.declare_dram_parameter("bias", [L, 3, D], F32, isOutput=False)
    idx_d = nc.declare_dram_parameter("idx", [P, IC], I16, isOutput=False)
    dstv_d = nc.declare_dram_parameter("dstv", [P, TT], F32, isOutput=False)
    deg_d = nc.declare_dram_parameter("deg", [P, W], F32, isOutput=False)
    iota_d = nc.declare_dram_parameter("iota", [P, P], F32, isOutput=False)
    curv_d = nc.declare_dram_parameter("curv", [P, 1], F32, isOutput=False)
    gcnt_d = nc.declare_dram_parameter("gcnt", [1, meta["NGATH"]],
                                       mybir.dt.int32, isOutput=False)
    out_d = nc.declare_dram_parameter("out", [SH, DX], F32, isOutput=True)

    emb_mid = [nc.dram_tensor(f"emb_mid{g}", [CH * P, DX], F32)
               for g in range(NG)]
    x_locA = [nc.dram_tensor(f"x_locA{l}", [RA, DX], XDT) for l in range(L)]
    x_locB = [nc.dram_tensor(f"x_locB{l}", [RB, DX], XDT) for l in range(L)]
    xfA = [nc.dram_tensor(f"xfA{l}", [NC_ * RA, DX], XDT, addr_space="Shared")
           for l in range(L)]
    xfB = [nc.dram_tensor(f"xfB{l}", [NC_ * RB, DX], XDT, addr_space="Shared")
           for l in range(L)]

    def rows3d(dram_ap, r0, ntiles):
        """DRAM rows [r0, r0+ntiles*128) as [128, ntiles, DX]."""
        return dram_ap[r0 : r0 + ntiles * P, :].rearrange(
            "(j p) d -> p j d", p=P)

    from contextlib import ExitStack
    with tile.TileContext(nc) as tc, ExitStack() as es:
        cpool = es.enter_context(tc.tile_pool(name="const", bufs=1))
        spool = es.enter_context(tc.tile_pool(name="work", bufs=2))
        gpool = es.enter_context(tc.tile_pool(name="gath", bufs=3))
        rpool = es.enter_context(tc.tile_pool(name="onehot", bufs=4))
        ppool = es.enter_context(tc.tile_pool(name="psum", bufs=2, space="PSUM"))
        ppool2 = es.enter_context(tc.tile_pool(name="psum2", bufs=2, space="PSUM"))

        # ---- constants ----
        iota_t = cpool.tile([P, P], F32)
        nc.sync.dma_start(out=iota_t[:], in_=iota_d[:])
        ident_t = cpool.tile([P, P], F32)
        make_identity(nc, ident_t[:])
        idx_t = cpool.tile([P, IC], I16)
        nc.sync.dma_start(out=idx_t[:], in_=idx_d[:])
        dstv_t = cpool.tile([P, TT], F32)
        nc.sync.dma_start(out=dstv_t[:], in_=dstv_d[:])
        deg_t = cpool.tile([P, W], F32)
        nc.sync.dma_start(out=deg_t[:], in_=deg_d[:])
        gcnt_t = cpool.tile([1, meta["NGATH"]], mybir.dt.int32)
        nc.sync.dma_start(out=gcnt_t[:], in_=gcnt_d[:])
        # zero the gather-buffer slots once: slots skipped by runtime-count
        # gathers keep stale data, and the one-hot matmul would turn virgin
        # (NaN) SBUF into 0*NaN=NaN despite the zero one-hot column.
        for _wi in range(3):
            wt = gpool.tile([P, Tmax, DX], XDT, tag="gb", name="gb")
            nc.vector.memset(wt[:], 0.0)
        ones1_t = cpool.tile([1, P], F32)
        nc.vector.memset(ones1_t[:], 1.0)
        wT_t = [[cpool.tile([D, D], F32, name=f"wT{l}{s}", tag=f"wT{l}{s}") for s in range(3)]
                for l in range(L)]
        bias_t = [[cpool.tile([1, D], F32, name=f"bias{l}{s}", tag=f"bias{l}{s}") for s in range(3)]
                  for l in range(L)]
        for l in range(L):
            for s in range(3):
                nc.sync.dma_start(out=wT_t[l][s][:], in_=wT_d[l, s])
                nc.sync.dma_start(out=bias_t[l][s][:], in_=bias_d[l, s][None, :])

        # recip = 1/max(deg,1)
        recip_t = cpool.tile([P, W], F32)
        nc.vector.tensor_scalar_max(out=recip_t[:], in0=deg_t[:], scalar1=1.0)
        nc.vector.reciprocal(out=recip_t[:], in_=recip_t[:])

        # curvature-derived scalars [128,1]
        curv_t = cpool.tile([P, 1], F32)
        nc.sync.dma_start(out=curv_t[:], in_=curv_d[:])
        sc_t = cpool.tile([P, 1], F32)       # sqrt(c)
        inv_sc_t = cpool.tile([P, 1], F32)   # 1/sqrt(c)
        nc.scalar.activation(out=sc_t[:], in_=curv_t[:],
                             func=mybir.ActivationFunctionType.Sqrt)
        nc.vector.reciprocal(out=inv_sc_t[:], in_=sc_t[:])

        def l2norm_chunk(xap, ntiles, eps, pfx=""):
            """In-place row-l2norm of [128, ntiles, 128] slice."""
            sq = spool.tile([P, ntiles, D], F32, tag=pfx + "sq", name="sq")
            nc.vector.tensor_tensor(out=sq[:], in0=xap, in1=xap,
                                    op=mybir.AluOpType.mult)
            n_ = spool.tile([P, ntiles], F32, tag=pfx + "nrm", name="nrm")
            nc.vector.tensor_reduce(out=n_[:], in_=sq[:],
                                    axis=mybir.AxisListType.X,
                                    op=mybir.AluOpType.add)
            nc.scalar.activation(out=n_[:], in_=n_[:],
                                 func=mybir.ActivationFunctionType.Sqrt)
            nc.vector.tensor_scalar_max(out=n_[:], in0=n_[:], scalar1=eps)
            nc.vector.reciprocal(out=n_[:], in_=n_[:])
            for j in range(ntiles):
                nc.vector.tensor_scalar_mul(out=xap[:, j, :], in0=xap[:, j, :],
                                            scalar1=n_[:, j : j + 1])

        def emit_build_chunk(l, g):
            r0 = g * CH * P
            ch = spool.tile([P, CH, DX], F32, tag="embch", name="embch")
            if l == 0:
                nc.sync.dma_start(out=ch[:], in_=rows3d(emb0_d[:], r0, CH))
            else:
                nc.sync.dma_start(out=ch[:], in_=rows3d(emb_mid[g][:], 0, CH))

            # tangent = logmap(b_emb): b cols [D, 2D)
            bpart = ch[:, :, D : 2 * D]
            sq = spool.tile([P, CH, D], F32, tag="sq", name="sq")
            nc.vector.tensor_tensor(out=sq[:], in0=bpart, in1=bpart,
                                    op=mybir.AluOpType.mult)
            n_ = spool.tile([P, CH], F32, tag="nrm", name="nrm")
            nc.vector.tensor_reduce(out=n_[:], in_=sq[:],
                                    axis=mybir.AxisListType.X,
                                    op=mybir.AluOpType.add)
            nc.scalar.activation(out=n_[:], in_=n_[:],
                                 func=mybir.ActivationFunctionType.Sqrt)
            nc.vector.tensor_scalar_max(out=n_[:], in0=n_[:], scalar1=EPS_LOG)
            a_ = spool.tile([P, CH], F32, tag="a_", name="a_")
            nc.vector.tensor_scalar(out=a_[:], in0=n_[:],
                                    scalar1=sc_t[:, 0:1], scalar2=None,
                                    op0=mybir.AluOpType.mult)
            t1 = spool.tile([P, CH], F32, tag="t1", name="t1")
            nc.scalar.activation(out=t1[:], in_=a_[:],
                                 func=mybir.ActivationFunctionType.Ln,
                                 bias=1.0, scale=1.0)
            t2 = spool.tile([P, CH], F32, tag="t2", name="t2")
            nc.scalar.activation(out=t2[:], in_=a_[:],
                                 func=mybir.ActivationFunctionType.Ln,
                                 bias=1.0, scale=-1.0)
            f_ = spool.tile([P, CH], F32, tag="f_", name="f_")
            nc.vector.tensor_tensor(out=f_[:], in0=t1[:], in1=t2[:],
                                    op=mybir.AluOpType.subtract)
            rn = spool.tile([P, CH], F32, tag="rn", name="rn")
            nc.vector.reciprocal(out=rn[:], in_=n_[:])
            nc.vector.tensor_tensor(out=f_[:], in0=f_[:], in1=rn[:],
                                    op=mybir.AluOpType.mult)
            nc.vector.tensor_scalar(out=f_[:], in0=f_[:],
                                    scalar1=inv_sc_t[:, 0:1], scalar2=None,
                                    op0=mybir.AluOpType.mult)
            tan = spool.tile([P, CH, D], F32, tag="tan", name="tan")
            for j in range(CH):
                nc.vector.tensor_scalar_mul(out=tan[:, j, :],
                                            in0=bpart[:, j, :],
                                            scalar1=f_[:, j : j + 1])

            sn = spool.tile([P, CH, D], F32, tag="sn", name="sn")
            nc.vector.tensor_copy(out=sn[:], in_=ch[:, :, 2 * D : 3 * D])
            l2norm_chunk(sn[:], CH, EPS_L2)

            xch = spool.tile([P, CH, DX], XDT, tag="xch", name="xch")
            for j in range(CH):
                ins_nm = (ch[:, j, 0:D], tan[:, j, :], sn[:, j, :])
                for sp_ in range(3):
                    tp = ppool2.tile([P, P], F32, space="PSUM", tag="tp",
                                     name="tp")
                    nc.tensor.transpose(out=tp[:], in_=ins_nm[sp_],
                                        identity=ident_t[:])
                    tsb = spool.tile([P, P], F32, tag="tsb", name="tsb")
                    nc.any.tensor_copy(out=tsb[:], in_=tp[:])
                    xp = ppool2.tile([P, D], F32, space="PSUM", tag="xp",
                                     name="xp")
                    nc.tensor.matmul(xp[:], lhsT=tsb[:], rhs=wT_t[l][sp_][:],
                                     start=True, stop=False)
                    nc.tensor.matmul(xp[:], lhsT=ones1_t[:],
                                     rhs=bias_t[l][sp_][:],
                                     start=False, stop=True)
                    nc.any.tensor_copy(out=xch[:, j, sp_ * D : (sp_ + 1) * D],
                                       in_=xp[:])
            l2norm_chunk(xch[:, :, 2 * D : 3 * D], CH, EPS_L2)
            if g < SPC:
                nc.sync.dma_start(out=rows3d(x_locA[l][:], r0, CH), in_=xch[:])
            else:
                nc.sync.dma_start(
                    out=rows3d(x_locB[l][:], r0 - RA, CH), in_=xch[:])

        def emit_allgather_a(l):
            nc.gpsimd.collective_compute(
                "AllGather", mybir.AluOpType.bypass,
                replica_groups=[list(range(NC_))],
                ins=[x_locA[l][:]], outs=[xfA[l][:]])

        def emit_allgather_b(l):
            nc.gpsimd.collective_compute(
                "AllGather", mybir.AluOpType.bypass,
                replica_groups=[list(range(NC_))],
                ins=[x_locB[l][:]], outs=[xfB[l][:]])

        gcnt_regs = [nc.gpsimd.alloc_register(f"gcnt_reg{i}")
                     for i in range(4)]
        _gri = [0]

        def gather_ordinal(w, half, off):
            # position of this sub-gather in host_prep's gcnt emission order
            gi = 0
            for w2 in range(w):
                gi += (K_lo[w2] + 1023) // 1024 + (K_hi[w2] + 1023) // 1024
            if half == 1:
                gi += (K_lo[w] + 1023) // 1024
            return gi + off // 1024

        def emit_agg_group(l, grp):
            pb = spool.tile([P, CH, DX], F32, tag="postbuf", name="postbuf")
            for wi in range(CH):
                w = grp * CH + wi
                tb = tile_base[w]
                gb = gpool.tile([P, Tmax, DX], XDT, tag="gb", name="gb")
                GMAX = 1024  # max idxs per dma_gather (descriptor ring cap)
                for half, K, t0_ in ((0, K_lo[w], 0), (1, K_hi[w], T_lo[w])):
                    src_ap = xfA[l][:] if half == 0 else xfB[l][:]
                    for off in range(0, K, GMAX):
                        kk = min(GMAX, K - off)
                        tstart = t0_ + off // P
                        cb = (tb + tstart) * P // 16
                        gi = gather_ordinal(w, half, off)
                        rv = gcnt_regs[_gri[0] % 4]
                        _gri[0] += 1
                        nc.gpsimd.reg_load(rv, gcnt_t[0:1, gi : gi + 1])
                        nc.gpsimd.dma_gather(
                            out_ap=gb[:, tstart : tstart + kk // P, :],
                            in_ap=src_ap,
                            idxs_ap=idx_t[:, cb : cb + kk // 16],
                            num_idxs=kk, num_idxs_reg=rv,
                            elem_size=DX)
                acc = ppool.tile([P, DX], F32, space="PSUM", tag="acc",
                                 name="acc")
                for t in range(T[w]):
                    r_ = rpool.tile([P, P], XDT, tag="r", name="r")
                    nc.vector.tensor_tensor(
                        out=r_[:],
                        in0=dstv_t[:, tb + t : tb + t + 1].to_broadcast([P, P]),
                        in1=iota_t[:],
                        op=mybir.AluOpType.is_equal)
                    nc.tensor.matmul(acc[:], lhsT=r_[:], rhs=gb[:, t, :],
                                     start=(t == 0), stop=(t == T[w] - 1))
                nc.vector.tensor_scalar_mul(out=pb[:, wi, :], in0=acc[:],
                                            scalar1=recip_t[:, w : w + 1])

            # post pointwise
            epart = pb[:, :, 0:D]
            tmp = spool.tile([P, CH, D], F32, tag="psq", name="psq")
            nc.vector.tensor_scalar_mul(out=tmp[:], in0=epart, scalar1=0.2)
            nc.vector.tensor_tensor(out=epart, in0=epart, in1=tmp[:],
                                    op=mybir.AluOpType.max)
            bpart = pb[:, :, D : 2 * D]
            nc.vector.tensor_tensor(out=tmp[:], in0=bpart, in1=bpart,
                                    op=mybir.AluOpType.mult)
            n_ = spool.tile([P, CH], F32, tag="pnrm", name="pnrm")
            nc.vector.tensor_reduce(out=n_[:], in_=tmp[:],
                                    axis=mybir.AxisListType.X,
                                    op=mybir.AluOpType.add)
            nc.scalar.activation(out=n_[:], in_=n_[:],
                                 func=mybir.ActivationFunctionType.Sqrt)
            nc.vector.tensor_scalar_max(out=n_[:], in0=n_[:], scalar1=EPS_LOG)
            a_ = spool.tile([P, CH], F32, tag="pa_", name="pa_")
            nc.vector.tensor_scalar(out=a_[:], in0=n_[:],
                                    scalar1=sc_t[:, 0:1], scalar2=None,
                                    op0=mybir.AluOpType.mult)
            th = spool.tile([P, CH], F32, tag="pt1", name="pt1")
            nc.scalar.activation(out=th[:], in_=a_[:],
                                 func=mybir.ActivationFunctionType.Tanh,
                                 scale=0.5)
            ra = spool.tile([P, CH], F32, tag="pt2", name="pt2")
            nc.vector.reciprocal(out=ra[:], in_=a_[:])
            nc.vector.tensor_tensor(out=th[:], in0=th[:], in1=ra[:],
                                    op=mybir.AluOpType.mult)
            for j in range(CH):
                nc.vector.tensor_scalar_mul(out=bpart[:, j, :],
                                            in0=bpart[:, j, :],
                                            scalar1=th[:, j : j + 1])
            l2norm_chunk(pb[:, :, 2 * D : 3 * D], CH, EPS_L2, pfx="p")

            if l == L - 1:
                nc.sync.dma_start(
                    out=rows3d(out_d[:], grp * CH * P, CH), in_=pb[:])
            else:
                nc.sync.dma_start(
                    out=rows3d(emb_mid[grp][:], 0, CH), in_=pb[:])

        # driver: interleave layer-(l+1) build chunks into layer-l agg
        # groups; AllGather half A fires once its chunks are built (half
        # tensors keep the WAR deps exact).
        for g in range(NG):
            emit_build_chunk(0, g)
            if g == SPC - 1:
                emit_allgather_a(0)
        emit_allgather_b(0)
        for g in range(NG):
            emit_agg_group(0, g)
            emit_build_chunk(1, g)
            if g == SPC - 1:
                emit_allgather_a(1)
        emit_allgather_b(1)
        for g in range(NG):
            emit_agg_group(1, g)

    return nc


def _build_in_maps(cfg, src, dst, e_emb, b_emb, s_emb, e_W, e_b, b_W, b_b,
                   s_W, s_b, b_curvature):
    N, SH, NC, DX, L, D = cfg.N, cfg.SH, cfg.NC, cfg.DX, cfg.L, cfg.D
    idx_all, dstv, deg_arr, gcnt, meta = _host_prep(cfg, src, dst)

    emb_full = np.zeros((cfg.NPAD, DX), np.float32)
    emb_full[:N, 0:D] = e_emb
    emb_full[:N, D:2 * D] = b_emb
    emb_full[:N, 2 * D:3 * D] = s_emb

    wT = np.stack([
        np.stack([e_W[l].T, b_W[l].T, s_W[l].T]) for l in range(L)
    ]).astype(np.float32)
    bias = np.stack([
        np.stack([e_b[l], b_b[l], s_b[l]]) for l in range(L)
    ]).astype(np.float32)

    iota = np.tile(np.arange(P, dtype=np.float32), (P, 1))
    curv = np.full((P, 1), np.float32(np.asarray(b_curvature).reshape(-1)[0]))

    in_maps = []
    for c in range(NC):
        in_maps.append({
            "emb0": np.ascontiguousarray(emb_full[c * SH:(c + 1) * SH]),
            "wT": wT,
            "bias": bias,
            "idx": np.ascontiguousarray(idx_all[c]),
            "dstv": np.ascontiguousarray(dstv[c]),
            "deg": np.ascontiguousarray(deg_arr[c]),
            "iota": iota,
            "curv": curv,
            "gcnt": np.ascontiguousarray(gcnt[c]),
        })
    return in_maps, meta


_LAST = {}


def run_kernel(inputs, trace=False):
    """Full pipeline; returns (results, exec_time_ns)."""
    from concourse.bass_utils import run_bass_kernel_spmd

    cfg = _Cfg(n_nodes=50000, n_edges=800000)
    src = np.asarray(inputs["src"], np.int32)
    dst = np.asarray(inputs["dst"], np.int32)
    in_maps, meta = _build_in_maps(
        cfg, src, dst,
        np.asarray(inputs["e_emb"], np.float32),
        np.asarray(inputs["b_emb"], np.float32),
        np.asarray(inputs["s_emb"], np.float32),
        np.asarray(inputs["e_W"], np.float32),
        np.asarray(inputs["e_b"], np.float32),
        np.asarray(inputs["b_W"], np.float32),
        np.asarray(inputs["b_b"], np.float32),
        np.asarray(inputs["s_W"], np.float32),
        np.asarray(inputs["s_b"], np.float32),
        np.asarray(inputs["b_curvature"], np.float32))

    # reuse the compiled program across calls when the graph layout matches
    gkey = (meta["TT"], hash(src.tobytes()) ^ hash(dst.tobytes()))
    nc = _LAST.get(gkey)
    if nc is None:
        nc = _build_nc(cfg, meta)
        nc.finalize()
        _split_excess_waits(nc)
        _LAST.clear()
        _LAST[gkey] = nc

    res = run_bass_kernel_spmd(nc, in_maps, core_ids=list(range(cfg.NC)),
                               trace=trace)
    full = np.concatenate([res.results[c]["out"] for c in range(cfg.NC)],
                          axis=0)[: cfg.N]
    D = cfg.D
    outs = (np.ascontiguousarray(full[:, 0:D]),
            np.ascontiguousarray(full[:, D:2 * D]),
            np.ascontiguousarray(full[:, 2 * D:3 * D]))
    return outs, res.exec_time_ns


def kernel(**inputs):
    outs, _ = run_kernel(inputs, trace=False)
    return outs



# revision 10
# speedup vs baseline: 1.3056x; 1.2639x over previous
"""Trainium2 Bass kernel for nn_APSDG (3-space GNN message passing).

8-core SPMD, dst-node sharding. Per layer:
  - own-shard node transform (logmap / l2norm pointwise + 128x128 matmuls)
    -> X_local [SH, 384] node-major
  - AllGather -> X_full [50176, 384] on every core
  - per 128-dst-node window: dma_gather X_full rows for the window's edges,
    one-hot matmul accumulation in PSUM (segment mean), pointwise post ops
    (LeakyReLU / expmap / l2norm) -> new embeddings.
Host side does integer-only edge prep (partitioning, padding, degree counts)
and layout; all float math runs on the NeuronCores.
"""
import sys

sys.path.insert(0, "/opt/trn_rl_repo")

import numpy as np

import concourse.bacc as bacc
import concourse.tile as tile
import concourse.mybir as mybir
from concourse.masks import make_identity

P = 128
F32 = mybir.dt.float32
I16 = mybir.dt.int16
EPS_LOG = 1e-10
EPS_L2 = 1e-12

# ---------------------------------------------------------------------------
# Workaround: this container's walrus codegen accepts only ONE sync-wait
# command per instruction, but Tile attaches several. Split the excess onto
# InstNoOps inserted before the instruction on the same engine (same-engine
# program order makes this equivalent for monotone sem-ge waits).
_ctr = [0]


def _split_excess_waits(nc, max_waits=1):
    def fresh():
        _ctr[0] += 1
        return f"WSPLIT-{_ctr[0]}"

    for f in nc.m.functions:
        for bb in f.blocks:
            insts = bb.instructions
            if not any(
                i.sync_info is not None and len(i.sync_info.on_wait) > max_waits
                for i in insts
            ):
                continue
            out = []
            for inst in insts:
                si = inst.sync_info
                if si is not None and len(si.on_wait) > max_waits:
                    waits = list(si.on_wait)
                    ge = [w for w in waits if "ge" in (w.wait_mode or "")]
                    eq = [w for w in waits if w not in ge]
                    keep = (eq + ge)[-max_waits:] if not eq else eq[-max_waits:]
                    hoist = [w for w in waits if w not in keep]
                    if len(keep) > max_waits:
                        raise RuntimeError(
                            f"{inst.name}: cannot split {len(eq)} eq-mode waits"
                        )
                    for i in range(0, len(hoist), max_waits):
                        nop = mybir.InstNoOp(name=fresh(), ins=[], outs=[])
                        nop.engine = inst.engine
                        nop.sync_info = mybir.SyncInfo(
                            on_wait=hoist[i : i + max_waits], on_update=[]
                        )
                        out.append(nop)
                    si.on_wait = keep
                out.append(inst)
            bb.instructions = out


# ---------------------------------------------------------------------------

P = 128
F32 = mybir.dt.float32
BF16 = mybir.dt.bfloat16
XDT = BF16  # storage dtype for gathered X (bf16: ~5e-3 rel err, gate 2e-2)
I16 = mybir.dt.int16
EPS_LOG = 1e-10   # log/exp map norm clamp (reference EPS)
EPS_L2 = 1e-12    # l2norm clamp


class _Cfg:
    def __init__(self, n_nodes, n_edges, n_cores=8, w_per_core=49, chunk=7,
                 base=32768, d=128, n_layers=2, split_chunks=None):
        self.N = n_nodes
        self.E = n_edges
        self.NC = n_cores
        self.W = w_per_core            # windows (128 dst nodes) per core
        self.CH = chunk                # node-tiles per processing chunk
        assert w_per_core % chunk == 0
        self.NG = w_per_core // chunk  # chunks/groups per core
        self.SH = w_per_core * P       # shard rows per core
        self.NPAD = self.SH * n_cores
        assert self.NPAD >= n_nodes
        self.BASE = base
        if split_chunks is None:
            split_chunks = max(1, (self.NG * 3) // 7)
        self.SPC = split_chunks            # chunks in the A half
        self.RA = split_chunks * chunk * P  # rows per core in half A
        self.RB = self.SH - self.RA
        assert n_cores * self.RA <= 32768 and n_cores * self.RB <= 32768
        self.D = d
        self.DX = 3 * d
        self.L = n_layers


def _host_prep(cfg, src, dst):
    """Integer-only edge prep. Returns per-core arrays + static meta."""
    NC, W, SH = cfg.NC, cfg.W, cfg.SH
    RA, RB = cfg.RA, cfg.RB
    src = np.asarray(src, np.int64)
    dst = np.asarray(dst, np.int64)

    core = dst // SH
    local = dst - core * SH
    win = local // P
    slot = local % P
    src_core = src // SH
    src_r = src - src_core * SH
    is_hi = src_r >= RA
    src_remap = np.where(is_hi, src_core * RB + (src_r - RA),
                         src_core * RA + src_r)

    # group edges by (core, window, is_hi): order by key, stable
    key = (core * W + win) * 2 + is_hi
    order = np.argsort(key, kind="stable")
    key_s = key[order]
    src_s = src_remap[order]
    slot_s = slot[order]
    # counts per (c, w, half)
    cnt = np.bincount(key_s, minlength=NC * W * 2).reshape(NC, W, 2)
    starts = np.zeros(NC * W * 2 + 1, np.int64)
    np.cumsum(cnt.reshape(-1), out=starts[1:])

    V_lo = cnt[:, :, 0]
    V_hi = cnt[:, :, 1]
    K_lo = ((V_lo.max(axis=0) + P - 1) // P) * P     # [W] uniform across cores
    K_hi = ((V_hi.max(axis=0) + P - 1) // P) * P
    T_lo = K_lo // P
    T_hi = K_hi // P
    T = T_lo + T_hi
    tile_base = np.zeros(W + 1, np.int64)
    np.cumsum(T, out=tile_base[1:])
    TT = int(tile_base[-1])                           # total tiles per core
    IC = TT * P // 16                                 # idx cols (int16 wrap)

    GMAX = 1024  # must match build_nc's per-gather split
    idx_all = np.zeros((NC, P, IC), np.int16)
    dstv = np.full((NC, P, TT), -1.0, np.float32)
    gcnt = [[] for _ in range(NC)]  # per-core valid count per sub-gather
    for c in range(NC):
        for w in range(W):
            for half in (0, 1):
                K = int(K_lo[w] if half == 0 else K_hi[w])
                if K == 0:
                    continue
                s0 = starts[(c * W + w) * 2 + half]
                s1 = starts[(c * W + w) * 2 + half + 1]
                e_src = src_s[s0:s1]
                e_slot = slot_s[s0:s1]
                V = len(e_src)
                idx_pad = np.full(K, -1, np.int64)
                idx_pad[:V] = e_src
                sl_pad = np.full(K, -1.0, np.float32)
                sl_pad[: len(e_slot)] = e_slot
                # per sub-gather: valid count; force >=1 valid (dummy idx 0,
                # its dstv slot stays -1 so it contributes nothing)
                for off in range(0, K, GMAX):
                    kk = min(GMAX, K - off)
                    v_here = min(max(V - off, 0), kk)
                    if v_here == 0:
                        idx_pad[off] = 0
                        v_here = 1
                    gcnt[c].append(v_here)
                tb = int(tile_base[w] + (0 if half == 0 else T_lo[w]))
                # idx wrap: index j -> [j%16, col_base + j//16], replicated x8
                wrap = idx_pad.reshape(-1, 16).T.astype(np.int16)  # [16, K/16]
                cb = tb * P // 16
                idx_all[c, :, cb : cb + K // 16] = np.tile(wrap, (8, 1))
                dstv[c, :, tb : tb + K // P] = sl_pad.reshape(-1, P).T
    gcnt = np.asarray(gcnt, np.int32)[:, None, :]  # [NC, 1, NGATH]

    deg = np.bincount(dst, minlength=cfg.NPAD).astype(np.float32)
    deg_arr = deg.reshape(NC, W, P).transpose(0, 2, 1).copy()  # [NC, 128, W]

    meta = dict(K_lo=K_lo.tolist(), K_hi=K_hi.tolist(),
                T_lo=T_lo.tolist(), T_hi=T_hi.tolist(), T=T.tolist(),
                tile_base=tile_base.tolist(), TT=TT, IC=IC,
                NGATH=int(gcnt.shape[2]))
    return idx_all, dstv, deg_arr, gcnt, meta


def _build_nc(cfg, meta):
    NC_, W, CH, NG, SH, NPAD, D, DX, L = (
        cfg.NC, cfg.W, cfg.CH, cfg.NG, cfg.SH, cfg.NPAD,
        cfg.D, cfg.DX, cfg.L)
    RA, RB, SPC = cfg.RA, cfg.RB, cfg.SPC
    TT, IC = meta["TT"], meta["IC"]
    K_lo, K_hi = meta["K_lo"], meta["K_hi"]
    T_lo, T_hi, T, tile_base = meta["T_lo"], meta["T_hi"], meta["T"], meta["tile_base"]
    Tmax = max(T)

    nc = bacc.Bacc("TRN2", target_bir_lowering=False, debug=False,
                   num_devices=NC_)

    emb0_d = nc.declare_dram_parameter("emb0", [SH, DX], F32, isOutput=False)
    wT_d = nc.declare_dram_parameter("wT", [L, 3, D, D], F32, isOutput=False)
    bias_d = nc.declare_dram_parameter("bias", [L, 3, P, D], F32,
                                       isOutput=False)
    idx_d = nc.declare_dram_parameter("idx", [P, IC], I16, isOutput=False)
    dstv_d = nc.declare_dram_parameter("dstv", [P, TT], F32, isOutput=False)
    deg_d = nc.declare_dram_parameter("deg", [P, W], F32, isOutput=False)
    iota_d = nc.declare_dram_parameter("iota", [P, P], F32, isOutput=False)
    curv_d = nc.declare_dram_parameter("curv", [P, 1], F32, isOutput=False)
    gcnt_d = nc.declare_dram_parameter("gcnt", [1, meta["NGATH"]],
                                       mybir.dt.int32, isOutput=False)
    out_d = nc.declare_dram_parameter("out", [SH, DX], F32, isOutput=True)

    emb_mid = [nc.dram_tensor(f"emb_mid{g}", [CH * P, DX], F32)
               for g in range(NG)]
    x_locA = [nc.dram_tensor(f"x_locA{l}", [RA, DX], XDT) for l in range(L)]
    x_locB = [nc.dram_tensor(f"x_locB{l}", [RB, DX], XDT) for l in range(L)]
    xfA = [nc.dram_tensor(f"xfA{l}", [NC_ * RA, DX], XDT, addr_space="Shared")
           for l in range(L)]
    xfB = [nc.dram_tensor(f"xfB{l}", [NC_ * RB, DX], XDT, addr_space="Shared")
           for l in range(L)]

    def rows3d(dram_ap, r0, ntiles):
        """DRAM rows [r0, r0+ntiles*128) as [128, ntiles, DX]."""
        return dram_ap[r0 : r0 + ntiles * P, :].rearrange(
            "(j p) d -> p j d", p=P)

    from contextlib import ExitStack
    with tile.TileContext(nc) as tc, ExitStack() as es:
        cpool = es.enter_context(tc.tile_pool(name="const", bufs=1))
        spool = es.enter_context(tc.tile_pool(name="work", bufs=2))
        gpool = es.enter_context(tc.tile_pool(name="gath", bufs=3))
        rpool = es.enter_context(tc.tile_pool(name="onehot", bufs=4))
        ppool = es.enter_context(tc.tile_pool(name="psum", bufs=2, space="PSUM"))
        ppool2 = es.enter_context(tc.tile_pool(name="psum2", bufs=2, space="PSUM"))

        # ---- constants ----
        iota_t = cpool.tile([P, P], F32)
        nc.sync.dma_start(out=iota_t[:], in_=iota_d[:])
        ident_t = cpool.tile([P, P], F32)
        make_identity(nc, ident_t[:])
        idx_t = cpool.tile([P, IC], I16)
        nc.sync.dma_start(out=idx_t[:], in_=idx_d[:])
        dstv_t = cpool.tile([P, TT], F32)
        nc.sync.dma_start(out=dstv_t[:], in_=dstv_d[:])
        deg_t = cpool.tile([P, W], F32)
        nc.sync.dma_start(out=deg_t[:], in_=deg_d[:])
        gcnt_t = cpool.tile([1, meta["NGATH"]], mybir.dt.int32)
        nc.sync.dma_start(out=gcnt_t[:], in_=gcnt_d[:])
        # zero the gather-buffer slots once: slots skipped by runtime-count
        # gathers keep stale data, and the one-hot matmul would turn virgin
        # (NaN) SBUF into 0*NaN=NaN despite the zero one-hot column.
        for _wi in range(3):
            wt = gpool.tile([P, Tmax, DX], XDT, tag="gb", name="gb")
            nc.vector.memset(wt[:], 0.0)
        # bf16 copies for the one-hot is_equal operands
        dstv_bf = cpool.tile([P, TT], BF16)
        nc.vector.tensor_copy(out=dstv_bf[:], in_=dstv_t[:])
        iota_bf = cpool.tile([P, P], BF16)
        nc.vector.tensor_copy(out=iota_bf[:], in_=iota_t[:])
        # weights: load fp32, cast once to bf16
        wT_f = [[cpool.tile([D, D], F32, name=f"wTf{l}{s}", tag=f"wTf{l}{s}")
                 for s in range(3)] for l in range(L)]
        wT_t = [[cpool.tile([D, D], BF16, name=f"wT{l}{s}", tag=f"wT{l}{s}")
                 for s in range(3)] for l in range(L)]
        bias_t = [[cpool.tile([P, D], F32, name=f"bias{l}{s}",
                              tag=f"bias{l}{s}") for s in range(3)]
                  for l in range(L)]
        for l in range(L):
            for s in range(3):
                nc.sync.dma_start(out=wT_f[l][s][:], in_=wT_d[l, s])
                nc.vector.tensor_copy(out=wT_t[l][s][:], in_=wT_f[l][s][:])
                nc.sync.dma_start(out=bias_t[l][s][:], in_=bias_d[l, s])

        # recip = 1/max(deg,1)
        recip_t = cpool.tile([P, W], F32)
        nc.vector.tensor_scalar_max(out=recip_t[:], in0=deg_t[:], scalar1=1.0)
        nc.vector.reciprocal(out=recip_t[:], in_=recip_t[:])

        # curvature-derived scalars [128,1]
        curv_t = cpool.tile([P, 1], F32)
        nc.sync.dma_start(out=curv_t[:], in_=curv_d[:])
        sc_t = cpool.tile([P, 1], F32)       # sqrt(c)
        inv_sc_t = cpool.tile([P, 1], F32)   # 1/sqrt(c)
        nc.scalar.activation(out=sc_t[:], in_=curv_t[:],
                             func=mybir.ActivationFunctionType.Sqrt)
        nc.vector.reciprocal(out=inv_sc_t[:], in_=sc_t[:])

        def l2norm_chunk(xap, ntiles, eps, pfx=""):
            """In-place row-l2norm of [128, ntiles, 128] slice."""
            sq = spool.tile([P, ntiles, D], F32, tag=pfx + "sq", name="sq")
            nc.vector.tensor_tensor(out=sq[:], in0=xap, in1=xap,
                                    op=mybir.AluOpType.mult)
            n_ = spool.tile([P, ntiles], F32, tag=pfx + "nrm", name="nrm")
            nc.vector.tensor_reduce(out=n_[:], in_=sq[:],
                                    axis=mybir.AxisListType.X,
                                    op=mybir.AluOpType.add)
            nc.scalar.activation(out=n_[:], in_=n_[:],
                                 func=mybir.ActivationFunctionType.Sqrt)
            nc.vector.tensor_scalar_max(out=n_[:], in0=n_[:], scalar1=eps)
            nc.vector.reciprocal(out=n_[:], in_=n_[:])
            for j in range(ntiles):
                nc.vector.tensor_scalar_mul(out=xap[:, j, :], in0=xap[:, j, :],
                                            scalar1=n_[:, j : j + 1])

        def emit_build_chunk(l, g):
            r0 = g * CH * P
            ch = spool.tile([P, CH, DX], F32, tag="embch", name="embch")
            if l == 0:
                nc.sync.dma_start(out=ch[:], in_=rows3d(emb0_d[:], r0, CH))
            else:
                nc.sync.dma_start(out=ch[:], in_=rows3d(emb_mid[g][:], 0, CH))

            # tangent = logmap(b_emb): b cols [D, 2D)
            bpart = ch[:, :, D : 2 * D]
            sq = spool.tile([P, CH, D], F32, tag="sq", name="sq")
            nc.vector.tensor_tensor(out=sq[:], in0=bpart, in1=bpart,
                                    op=mybir.AluOpType.mult)
            n_ = spool.tile([P, CH], F32, tag="nrm", name="nrm")
            nc.vector.tensor_reduce(out=n_[:], in_=sq[:],
                                    axis=mybir.AxisListType.X,
                                    op=mybir.AluOpType.add)
            nc.scalar.activation(out=n_[:], in_=n_[:],
                                 func=mybir.ActivationFunctionType.Sqrt)
            nc.vector.tensor_scalar_max(out=n_[:], in0=n_[:], scalar1=EPS_LOG)
            a_ = spool.tile([P, CH], F32, tag="a_", name="a_")
            nc.vector.tensor_scalar(out=a_[:], in0=n_[:],
                                    scalar1=sc_t[:, 0:1], scalar2=None,
                                    op0=mybir.AluOpType.mult)
            t1 = spool.tile([P, CH], F32, tag="t1", name="t1")
            nc.scalar.activation(out=t1[:], in_=a_[:],
                                 func=mybir.ActivationFunctionType.Ln,
                                 bias=1.0, scale=1.0)
            t2 = spool.tile([P, CH], F32, tag="t2", name="t2")
            nc.scalar.activation(out=t2[:], in_=a_[:],
                                 func=mybir.ActivationFunctionType.Ln,
                                 bias=1.0, scale=-1.0)
            f_ = spool.tile([P, CH], F32, tag="f_", name="f_")
            nc.vector.tensor_tensor(out=f_[:], in0=t1[:], in1=t2[:],
                                    op=mybir.AluOpType.subtract)
            rn = spool.tile([P, CH], F32, tag="rn", name="rn")
            nc.vector.reciprocal(out=rn[:], in_=n_[:])
            nc.vector.tensor_tensor(out=f_[:], in0=f_[:], in1=rn[:],
                                    op=mybir.AluOpType.mult)
            nc.vector.tensor_scalar(out=f_[:], in0=f_[:],
                                    scalar1=inv_sc_t[:, 0:1], scalar2=None,
                                    op0=mybir.AluOpType.mult)
            tan = spool.tile([P, CH, D], F32, tag="tan", name="tan")
            for j in range(CH):
                nc.vector.tensor_scalar_mul(out=tan[:, j, :],
                                            in0=bpart[:, j, :],
                                            scalar1=f_[:, j : j + 1])

            sn = spool.tile([P, CH, D], F32, tag="sn", name="sn")
            nc.vector.tensor_copy(out=sn[:], in_=ch[:, :, 2 * D : 3 * D])
            l2norm_chunk(sn[:], CH, EPS_L2)

            xch = spool.tile([P, CH, DX], XDT, tag="xch", name="xch")
            for j in range(CH):
                ins_nm = (ch[:, j, 0:D], tan[:, j, :], sn[:, j, :])
                for sp_ in range(3):
                    tp = ppool2.tile([P, P], F32, space="PSUM", tag="tp",
                                     name="tp")
                    nc.tensor.transpose(out=tp[:], in_=ins_nm[sp_],
                                        identity=ident_t[:])
                    tsb = spool.tile([P, P], BF16, tag="tsb", name="tsb")
                    nc.any.tensor_copy(out=tsb[:], in_=tp[:])
                    xp = ppool2.tile([P, D], F32, space="PSUM", tag="xp",
                                     name="xp")
                    nc.tensor.matmul(xp[:], lhsT=tsb[:], rhs=wT_t[l][sp_][:],
                                     start=True, stop=True)
                    # bias add fused into the PSUM -> SBUF copy
                    nc.vector.tensor_tensor(
                        out=xch[:, j, sp_ * D : (sp_ + 1) * D],
                        in0=xp[:], in1=bias_t[l][sp_][:],
                        op=mybir.AluOpType.add)
            l2norm_chunk(xch[:, :, 2 * D : 3 * D], CH, EPS_L2)
            if g < SPC:
                nc.sync.dma_start(out=rows3d(x_locA[l][:], r0, CH), in_=xch[:])
            else:
                nc.sync.dma_start(
                    out=rows3d(x_locB[l][:], r0 - RA, CH), in_=xch[:])

        def emit_allgather_a(l):
            nc.gpsimd.collective_compute(
                "AllGather", mybir.AluOpType.bypass,
                replica_groups=[list(range(NC_))],
                ins=[x_locA[l][:]], outs=[xfA[l][:]])

        def emit_allgather_b(l):
            nc.gpsimd.collective_compute(
                "AllGather", mybir.AluOpType.bypass,
                replica_groups=[list(range(NC_))],
                ins=[x_locB[l][:]], outs=[xfB[l][:]])

        gcnt_regs = [nc.gpsimd.alloc_register(f"gcnt_reg{i}")
                     for i in range(4)]
        _gri = [0]

        def gather_ordinal(w, half, off):
            # position of this sub-gather in host_prep's gcnt emission order
            gi = 0
            for w2 in range(w):
                gi += (K_lo[w2] + 1023) // 1024 + (K_hi[w2] + 1023) // 1024
            if half == 1:
                gi += (K_lo[w] + 1023) // 1024
            return gi + off // 1024

        def emit_agg_group(l, grp):
            pb = spool.tile([P, CH, DX], F32, tag="postbuf", name="postbuf")
            for wi in range(CH):
                w = grp * CH + wi
                tb = tile_base[w]
                gb = gpool.tile([P, Tmax, DX], XDT, tag="gb", name="gb")
                GMAX = 1024  # max idxs per dma_gather (descriptor ring cap)
                for half, K, t0_ in ((0, K_lo[w], 0), (1, K_hi[w], T_lo[w])):
                    src_ap = xfA[l][:] if half == 0 else xfB[l][:]
                    for off in range(0, K, GMAX):
                        kk = min(GMAX, K - off)
                        tstart = t0_ + off // P
                        cb = (tb + tstart) * P // 16
                        gi = gather_ordinal(w, half, off)
                        rv = gcnt_regs[_gri[0] % 4]
                        _gri[0] += 1
                        nc.gpsimd.reg_load(rv, gcnt_t[0:1, gi : gi + 1])
                        nc.gpsimd.dma_gather(
                            out_ap=gb[:, tstart : tstart + kk // P, :],
                            in_ap=src_ap,
                            idxs_ap=idx_t[:, cb : cb + kk // 16],
                            num_idxs=kk, num_idxs_reg=rv,
                            elem_size=DX)
                acc = ppool.tile([P, DX], F32, space="PSUM", tag="acc",
                                 name="acc")
                for t in range(T[w]):
                    r_ = rpool.tile([P, P], XDT, tag="r", name="r")
                    nc.vector.tensor_tensor(
                        out=r_[:],
                        in0=dstv_bf[:, tb + t : tb + t + 1].to_broadcast([P, P]),
                        in1=iota_bf[:],
                        op=mybir.AluOpType.is_equal)
                    nc.tensor.matmul(acc[:], lhsT=r_[:], rhs=gb[:, t, :],
                                     start=(t == 0), stop=(t == T[w] - 1))
                nc.vector.tensor_scalar_mul(out=pb[:, wi, :], in0=acc[:],
                                            scalar1=recip_t[:, w : w + 1])

            # post pointwise
            epart = pb[:, :, 0:D]
            tmp = spool.tile([P, CH, D], F32, tag="psq", name="psq")
            nc.vector.tensor_scalar_mul(out=tmp[:], in0=epart, scalar1=0.2)
            nc.vector.tensor_tensor(out=epart, in0=epart, in1=tmp[:],
                                    op=mybir.AluOpType.max)
            bpart = pb[:, :, D : 2 * D]
            nc.vector.tensor_tensor(out=tmp[:], in0=bpart, in1=bpart,
                                    op=mybir.AluOpType.mult)
            n_ = spool.tile([P, CH], F32, tag="pnrm", name="pnrm")
            nc.vector.tensor_reduce(out=n_[:], in_=tmp[:],
                                    axis=mybir.AxisListType.X,
                                    op=mybir.AluOpType.add)
            nc.scalar.activation(out=n_[:], in_=n_[:],
                                 func=mybir.ActivationFunctionType.Sqrt)
            nc.vector.tensor_scalar_max(out=n_[:], in0=n_[:], scalar1=EPS_LOG)
            a_ = spool.tile([P, CH], F32, tag="pa_", name="pa_")
            nc.vector.tensor_scalar(out=a_[:], in0=n_[:],
                                    scalar1=sc_t[:, 0:1], scalar2=None,
                                    op0=mybir.AluOpType.mult)
            th = spool.tile([P, CH], F32, tag="pt1", name="pt1")
            nc.scalar.activation(out=th[:], in_=a_[:],
                                 func=mybir.ActivationFunctionType.Tanh,
                                 scale=0.5)
            ra = spool.tile([P, CH], F32, tag="pt2", name="pt2")
            nc.vector.reciprocal(out=ra[:], in_=a_[:])
            nc.vector.tensor_tensor(out=th[:], in0=th[:], in1=ra[:],
                                    op=mybir.AluOpType.mult)
            for j in range(CH):
                nc.vector.tensor_scalar_mul(out=bpart[:, j, :],
                                            in0=bpart[:, j, :],
                                            scalar1=th[:, j : j + 1])
            l2norm_chunk(pb[:, :, 2 * D : 3 * D], CH, EPS_L2, pfx="p")

            if l == L - 1:
                nc.sync.dma_start(
                    out=rows3d(out_d[:], grp * CH * P, CH), in_=pb[:])
            else:
                nc.sync.dma_start(
                    out=rows3d(emb_mid[grp][:], 0, CH), in_=pb[:])

        # driver: interleave layer-(l+1) build chunks into layer-l agg
        # groups; AllGather half A fires once its chunks are built (half
        # tensors keep the WAR deps exact).
        for g in range(NG):
            emit_build_chunk(0, g)
            if g == SPC - 1:
                emit_allgather_a(0)
        emit_allgather_b(0)
        for g in range(NG):
            emit_agg_group(0, g)
            emit_build_chunk(1, g)
            if g == SPC - 1:
                emit_allgather_a(1)
        emit_allgather_b(1)
        for g in range(NG):
            emit_agg_group(1, g)

    return nc


def _build_in_maps(cfg, src, dst, e_emb, b_emb, s_emb, e_W, e_b, b_W, b_b,
                   s_W, s_b, b_curvature):
    N, SH, NC, DX, L, D = cfg.N, cfg.SH, cfg.NC, cfg.DX, cfg.L, cfg.D
    idx_all, dstv, deg_arr, gcnt, meta = _host_prep(cfg, src, dst)

    emb_full = np.zeros((cfg.NPAD, DX), np.float32)
    emb_full[:N, 0:D] = e_emb
    emb_full[:N, D:2 * D] = b_emb
    emb_full[:N, 2 * D:3 * D] = s_emb

    wT = np.stack([
        np.stack([e_W[l].T, b_W[l].T, s_W[l].T]) for l in range(L)
    ]).astype(np.float32)
    bias = np.stack([
        np.stack([e_b[l], b_b[l], s_b[l]]) for l in range(L)
    ]).astype(np.float32)
    # broadcast per-feature bias across the 128 node partitions
    bias_bc = np.ascontiguousarray(
        np.broadcast_to(bias[:, :, None, :], (L, 3, P, D)), dtype=np.float32)

    iota = np.tile(np.arange(P, dtype=np.float32), (P, 1))
    curv = np.full((P, 1), np.float32(np.asarray(b_curvature).reshape(-1)[0]))

    in_maps = []
    for c in range(NC):
        in_maps.append({
            "emb0": np.ascontiguousarray(emb_full[c * SH:(c + 1) * SH]),
            "wT": wT,
            "bias": bias_bc,
            "idx": np.ascontiguousarray(idx_all[c]),
            "dstv": np.ascontiguousarray(dstv[c]),
            "deg": np.ascontiguousarray(deg_arr[c]),
            "iota": iota,
            "curv": curv,
            "gcnt": np.ascontiguousarray(gcnt[c]),
        })
    return in_maps, meta


_LAST = {}


def run_kernel(inputs, trace=False):
    """Full pipeline; returns (results, exec_time_ns)."""
    from concourse.bass_utils import run_bass_kernel_spmd

    cfg = _Cfg(n_nodes=50000, n_edges=800000)
    src = np.asarray(inputs["src"], np.int32)
    dst = np.asarray(inputs["dst"], np.int32)
    in_maps, meta = _build_in_maps(
        cfg, src, dst,
        np.asarray(inputs["e_emb"], np.float32),
        np.asarray(inputs["b_emb"], np.float32),
        np.asarray(inputs["s_emb"], np.float32),
        np.asarray(inputs["e_W"], np.float32),
        np.asarray(inputs["e_b"], np.float32),
        np.asarray(inputs["b_W"], np.float32),
        np.asarray(inputs["b_b"], np.float32),
        np.asarray(inputs["s_W"], np.float32),
        np.asarray(inputs["s_b"], np.float32),
        np.asarray(inputs["b_curvature"], np.float32))

    # reuse the compiled program across calls when the graph layout matches
    gkey = (meta["TT"], hash(src.tobytes()) ^ hash(dst.tobytes()))
    nc = _LAST.get(gkey)
    if nc is None:
        nc = _build_nc(cfg, meta)
        nc.finalize()
        _split_excess_waits(nc)
        _LAST.clear()
        _LAST[gkey] = nc

    res = run_bass_kernel_spmd(nc, in_maps, core_ids=list(range(cfg.NC)),
                               trace=trace)
    full = np.concatenate([res.results[c]["out"] for c in range(cfg.NC)],
                          axis=0)[: cfg.N]
    D = cfg.D
    outs = (np.ascontiguousarray(full[:, 0:D]),
            np.ascontiguousarray(full[:, D:2 * D]),
            np.ascontiguousarray(full[:, 2 * D:3 * D]))
    return outs, res.exec_time_ns


def kernel(**inputs):
    outs, _ = run_kernel(inputs, trace=False)
    return outs



# revision 15
# speedup vs baseline: 2.2054x; 1.6892x over previous
"""Trainium2 Bass kernel for nn_APSDG (3-space GNN message passing).

8-core SPMD, dst-node sharding. Per layer:
  - own-shard node transform (logmap / l2norm pointwise + 128x128 matmuls)
    -> X_local [SH, 384] node-major
  - AllGather -> X_full [50176, 384] on every core
  - per 128-dst-node window: dma_gather X_full rows for the window's edges,
    one-hot matmul accumulation in PSUM (segment mean), pointwise post ops
    (LeakyReLU / expmap / l2norm) -> new embeddings.
Host side does integer-only edge prep (partitioning, padding, degree counts)
and layout; all float math runs on the NeuronCores.
"""
import sys

sys.path.insert(0, "/opt/trn_rl_repo")

import numpy as np

import concourse.bacc as bacc
import concourse.tile as tile
import concourse.mybir as mybir
from concourse.masks import make_identity

P = 128
F32 = mybir.dt.float32
I16 = mybir.dt.int16
EPS_LOG = 1e-10
EPS_L2 = 1e-12

# ---------------------------------------------------------------------------
# Workaround: this container's walrus codegen accepts only ONE sync-wait
# command per instruction, but Tile attaches several. Split the excess onto
# InstNoOps inserted before the instruction on the same engine (same-engine
# program order makes this equivalent for monotone sem-ge waits).
_ctr = [0]


def _split_excess_waits(nc, max_waits=1):
    def fresh():
        _ctr[0] += 1
        return f"WSPLIT-{_ctr[0]}"

    for f in nc.m.functions:
        for bb in f.blocks:
            insts = bb.instructions
            if not any(
                i.sync_info is not None and len(i.sync_info.on_wait) > max_waits
                for i in insts
            ):
                continue
            out = []
            for inst in insts:
                si = inst.sync_info
                if si is not None and len(si.on_wait) > max_waits:
                    waits = list(si.on_wait)
                    ge = [w for w in waits if "ge" in (w.wait_mode or "")]
                    eq = [w for w in waits if w not in ge]
                    keep = (eq + ge)[-max_waits:] if not eq else eq[-max_waits:]
                    hoist = [w for w in waits if w not in keep]
                    if len(keep) > max_waits:
                        raise RuntimeError(
                            f"{inst.name}: cannot split {len(eq)} eq-mode waits"
                        )
                    for i in range(0, len(hoist), max_waits):
                        nop = mybir.InstNoOp(name=fresh(), ins=[], outs=[])
                        nop.engine = inst.engine
                        nop.sync_info = mybir.SyncInfo(
                            on_wait=hoist[i : i + max_waits], on_update=[]
                        )
                        out.append(nop)
                    si.on_wait = keep
                out.append(inst)
            bb.instructions = out


# ---------------------------------------------------------------------------

P = 128
F32 = mybir.dt.float32
BF16 = mybir.dt.bfloat16
XDT = BF16  # storage dtype for gathered X (bf16: ~5e-3 rel err, gate 2e-2)
I16 = mybir.dt.int16
EPS_LOG = 1e-10   # log/exp map norm clamp (reference EPS)
EPS_L2 = 1e-12    # l2norm clamp


class _Cfg:
    def __init__(self, n_nodes, n_edges, n_cores=8, w_per_core=49, chunk=7,
                 base=32768, d=128, n_layers=2, split_chunks=None):
        self.N = n_nodes
        self.E = n_edges
        self.NC = n_cores
        self.W = w_per_core            # windows (128 dst nodes) per core
        self.CH = chunk                # node-tiles per processing chunk
        assert w_per_core % chunk == 0
        self.NG = w_per_core // chunk  # chunks/groups per core
        self.SH = w_per_core * P       # shard rows per core
        self.NPAD = self.SH * n_cores
        assert self.NPAD >= n_nodes
        self.BASE = base
        if split_chunks is None:
            split_chunks = max(1, (self.NG * 3) // 7)
        self.SPC = split_chunks            # chunks in the A half
        self.RA = split_chunks * chunk * P  # rows per core in half A
        self.RB = self.SH - self.RA
        assert n_cores * self.RA <= 32768 and n_cores * self.RB <= 32768
        self.D = d
        self.DX = 3 * d
        self.L = n_layers


def _host_prep(cfg, src, dst):
    """Integer-only edge prep. Returns per-core arrays + static meta."""
    NC, W, SH = cfg.NC, cfg.W, cfg.SH
    RA, RB = cfg.RA, cfg.RB
    src = np.asarray(src, np.int64)
    dst = np.asarray(dst, np.int64)

    core = dst // SH
    local = dst - core * SH
    win = local // P
    slot = local % P
    src_core = src // SH
    src_r = src - src_core * SH
    is_hi = src_r >= RA
    src_remap = np.where(is_hi, src_core * RB + (src_r - RA),
                         src_core * RA + src_r)

    # group edges by (core, window, is_hi): order by key, stable
    key = (core * W + win) * 2 + is_hi
    order = np.argsort(key, kind="stable")
    key_s = key[order]
    src_s = src_remap[order]
    slot_s = slot[order]
    # counts per (c, w, half)
    cnt = np.bincount(key_s, minlength=NC * W * 2).reshape(NC, W, 2)
    starts = np.zeros(NC * W * 2 + 1, np.int64)
    np.cumsum(cnt.reshape(-1), out=starts[1:])

    V_lo = cnt[:, :, 0]
    V_hi = cnt[:, :, 1]
    K_lo = ((V_lo.max(axis=0) + P - 1) // P) * P     # [W] uniform across cores
    K_hi = ((V_hi.max(axis=0) + P - 1) // P) * P
    T_lo = K_lo // P
    T_hi = K_hi // P
    T = T_lo + T_hi
    tile_base = np.zeros(W + 1, np.int64)
    np.cumsum(T, out=tile_base[1:])
    TT = int(tile_base[-1])                           # total tiles per core
    IC = TT * P // 16                                 # idx cols (int16 wrap)

    GMAX = 1024  # must match build_nc's per-gather split
    idx_all = np.zeros((NC, P, IC), np.int16)
    dstv = np.full((NC, P, TT), -1.0, np.float32)
    gcnt = [[] for _ in range(NC)]  # per-core valid count per sub-gather
    for c in range(NC):
        for w in range(W):
            for half in (0, 1):
                K = int(K_lo[w] if half == 0 else K_hi[w])
                if K == 0:
                    continue
                s0 = starts[(c * W + w) * 2 + half]
                s1 = starts[(c * W + w) * 2 + half + 1]
                e_src = src_s[s0:s1]
                e_slot = slot_s[s0:s1]
                V = len(e_src)
                idx_pad = np.full(K, -1, np.int64)
                idx_pad[:V] = e_src
                sl_pad = np.full(K, -1.0, np.float32)
                sl_pad[: len(e_slot)] = e_slot
                # per sub-gather: valid count; force >=1 valid (dummy idx 0,
                # its dstv slot stays -1 so it contributes nothing)
                for off in range(0, K, GMAX):
                    kk = min(GMAX, K - off)
                    v_here = min(max(V - off, 0), kk)
                    if v_here == 0:
                        idx_pad[off] = 0
                        v_here = 1
                    gcnt[c].append(v_here)
                tb = int(tile_base[w] + (0 if half == 0 else T_lo[w]))
                # idx wrap: index j -> [j%16, col_base + j//16], replicated x8
                wrap = idx_pad.reshape(-1, 16).T.astype(np.int16)  # [16, K/16]
                cb = tb * P // 16
                idx_all[c, :, cb : cb + K // 16] = np.tile(wrap, (8, 1))
                dstv[c, :, tb : tb + K // P] = sl_pad.reshape(-1, P).T
    gcnt = np.asarray(gcnt, np.int32)[:, None, :]  # [NC, 1, NGATH]

    deg = np.bincount(dst, minlength=cfg.NPAD).astype(np.float32)
    deg_arr = deg.reshape(NC, W, P).transpose(0, 2, 1).copy()  # [NC, 128, W]

    meta = dict(K_lo=K_lo.tolist(), K_hi=K_hi.tolist(),
                T_lo=T_lo.tolist(), T_hi=T_hi.tolist(), T=T.tolist(),
                tile_base=tile_base.tolist(), TT=TT, IC=IC,
                NGATH=int(gcnt.shape[2]))
    return idx_all, dstv, deg_arr, gcnt, meta


def _build_nc(cfg, meta):
    NC_, W, CH, NG, SH, NPAD, D, DX, L = (
        cfg.NC, cfg.W, cfg.CH, cfg.NG, cfg.SH, cfg.NPAD,
        cfg.D, cfg.DX, cfg.L)
    RA, RB, SPC = cfg.RA, cfg.RB, cfg.SPC
    TT, IC = meta["TT"], meta["IC"]
    K_lo, K_hi = meta["K_lo"], meta["K_hi"]
    T_lo, T_hi, T, tile_base = meta["T_lo"], meta["T_hi"], meta["T"], meta["tile_base"]
    Tmax = max(T)

    nc = bacc.Bacc("TRN2", target_bir_lowering=False, debug=False,
                   num_devices=NC_, num_swdge_queues=4)

    emb0_d = nc.declare_dram_parameter("emb0", [SH, DX], F32, isOutput=False)
    wT_d = nc.declare_dram_parameter("wT", [L, 3, D, D], F32, isOutput=False)
    bias_d = nc.declare_dram_parameter("bias", [L, 3, P, D], F32,
                                       isOutput=False)
    idx_d = nc.declare_dram_parameter("idx", [P, IC], I16, isOutput=False)
    dstv_d = nc.declare_dram_parameter("dstv", [P, TT], F32, isOutput=False)
    deg_d = nc.declare_dram_parameter("deg", [P, W], F32, isOutput=False)
    iota_d = nc.declare_dram_parameter("iota", [P, P], F32, isOutput=False)
    curv_d = nc.declare_dram_parameter("curv", [P, 1], F32, isOutput=False)
    gcnt_d = nc.declare_dram_parameter("gcnt", [1, meta["NGATH"]],
                                       mybir.dt.int32, isOutput=False)
    out_d = nc.declare_dram_parameter("out", [SH, DX], F32, isOutput=True)

    emb_mid = [nc.dram_tensor(f"emb_mid{g}", [CH * P, DX], F32)
               for g in range(NG)]
    x_locA = [nc.dram_tensor(f"x_locA{l}", [RA, DX], XDT) for l in range(L)]
    x_locB = [nc.dram_tensor(f"x_locB{l}", [RB, DX], XDT) for l in range(L)]
    xfA = [nc.dram_tensor(f"xfA{l}", [NC_ * RA, DX], XDT, addr_space="Shared")
           for l in range(L)]
    xfB = [nc.dram_tensor(f"xfB{l}", [NC_ * RB, DX], XDT, addr_space="Shared")
           for l in range(L)]

    def rows3d(dram_ap, r0, ntiles):
        """DRAM rows [r0, r0+ntiles*128) as [128, ntiles, DX]."""
        return dram_ap[r0 : r0 + ntiles * P, :].rearrange(
            "(j p) d -> p j d", p=P)

    from contextlib import ExitStack
    with tile.TileContext(nc) as tc, ExitStack() as es:
        cpool = es.enter_context(tc.tile_pool(name="const", bufs=1))
        spool = es.enter_context(tc.tile_pool(name="work", bufs=2))
        gpool = es.enter_context(tc.tile_pool(name="gath", bufs=4))
        rpool = es.enter_context(tc.tile_pool(name="onehot", bufs=4))
        ppool = es.enter_context(tc.tile_pool(name="psum", bufs=4, space="PSUM"))
        ppool2 = es.enter_context(tc.tile_pool(name="psum2", bufs=2, space="PSUM"))

        # ---- constants ----
        iota_t = cpool.tile([P, P], F32)
        nc.sync.dma_start(out=iota_t[:], in_=iota_d[:])
        ident_t = cpool.tile([P, P], F32)
        make_identity(nc, ident_t[:])
        idx_t = cpool.tile([P, IC], I16)
        nc.sync.dma_start(out=idx_t[:], in_=idx_d[:])
        dstv_t = cpool.tile([P, TT], F32)
        nc.sync.dma_start(out=dstv_t[:], in_=dstv_d[:])
        deg_t = cpool.tile([P, W], F32)
        nc.sync.dma_start(out=deg_t[:], in_=deg_d[:])
        gcnt_t = cpool.tile([1, meta["NGATH"]], mybir.dt.int32)
        nc.sync.dma_start(out=gcnt_t[:], in_=gcnt_d[:])
        # zero the gather-buffer slots once: slots skipped by runtime-count
        # gathers keep stale data, and the one-hot matmul would turn virgin
        # (NaN) SBUF into 0*NaN=NaN despite the zero one-hot column.
        for _wi in range(4):
            wt = gpool.tile([P, Tmax, DX], XDT, tag="gb", name="gb")
            nc.vector.memset(wt[:], 0.0)
        # bf16 copies for the one-hot is_equal operands
        dstv_bf = cpool.tile([P, TT], BF16)
        nc.vector.tensor_copy(out=dstv_bf[:], in_=dstv_t[:])
        iota_bf = cpool.tile([P, P], BF16)
        nc.vector.tensor_copy(out=iota_bf[:], in_=iota_t[:])
        # weights: load fp32, cast once to bf16
        wT_f = [[cpool.tile([D, D], F32, name=f"wTf{l}{s}", tag=f"wTf{l}{s}")
                 for s in range(3)] for l in range(L)]
        wT_t = [[cpool.tile([D, D], BF16, name=f"wT{l}{s}", tag=f"wT{l}{s}")
                 for s in range(3)] for l in range(L)]
        bias_t = [[cpool.tile([P, D], F32, name=f"bias{l}{s}",
                              tag=f"bias{l}{s}") for s in range(3)]
                  for l in range(L)]
        for l in range(L):
            for s in range(3):
                nc.sync.dma_start(out=wT_f[l][s][:], in_=wT_d[l, s])
                nc.vector.tensor_copy(out=wT_t[l][s][:], in_=wT_f[l][s][:])
                nc.sync.dma_start(out=bias_t[l][s][:], in_=bias_d[l, s])

        # recip = 1/max(deg,1)
        recip_t = cpool.tile([P, W], F32)
        nc.vector.tensor_scalar_max(out=recip_t[:], in0=deg_t[:], scalar1=1.0)
        nc.vector.reciprocal(out=recip_t[:], in_=recip_t[:])

        # curvature-derived scalars [128,1]
        curv_t = cpool.tile([P, 1], F32)
        nc.sync.dma_start(out=curv_t[:], in_=curv_d[:])
        sc_t = cpool.tile([P, 1], F32)       # sqrt(c)
        inv_sc_t = cpool.tile([P, 1], F32)   # 1/sqrt(c)
        nc.scalar.activation(out=sc_t[:], in_=curv_t[:],
                             func=mybir.ActivationFunctionType.Sqrt)
        nc.vector.reciprocal(out=inv_sc_t[:], in_=sc_t[:])

        def l2norm_chunk(xap, ntiles, eps, pfx=""):
            """In-place row-l2norm of [128, ntiles, 128] slice."""
            sq = spool.tile([P, ntiles, D], F32, tag=pfx + "sq", name="sq")
            nc.vector.tensor_tensor(out=sq[:], in0=xap, in1=xap,
                                    op=mybir.AluOpType.mult)
            n_ = spool.tile([P, ntiles], F32, tag=pfx + "nrm", name="nrm")
            nc.vector.tensor_reduce(out=n_[:], in_=sq[:],
                                    axis=mybir.AxisListType.X,
                                    op=mybir.AluOpType.add)
            nc.scalar.activation(out=n_[:], in_=n_[:],
                                 func=mybir.ActivationFunctionType.Sqrt)
            nc.vector.tensor_scalar_max(out=n_[:], in0=n_[:], scalar1=eps)
            nc.vector.reciprocal(out=n_[:], in_=n_[:])
            for j in range(ntiles):
                nc.vector.tensor_scalar_mul(out=xap[:, j, :], in0=xap[:, j, :],
                                            scalar1=n_[:, j : j + 1])

        def emit_build_chunk(l, g):
            r0 = g * CH * P
            ch = spool.tile([P, CH, DX], F32, tag="embch", name="embch")
            if l == 0:
                nc.sync.dma_start(out=ch[:], in_=rows3d(emb0_d[:], r0, CH))
            else:
                nc.sync.dma_start(out=ch[:], in_=rows3d(emb_mid[g][:], 0, CH))

            # tangent = logmap(b_emb): b cols [D, 2D)
            bpart = ch[:, :, D : 2 * D]
            sq = spool.tile([P, CH, D], F32, tag="sq", name="sq")
            nc.vector.tensor_tensor(out=sq[:], in0=bpart, in1=bpart,
                                    op=mybir.AluOpType.mult)
            n_ = spool.tile([P, CH], F32, tag="nrm", name="nrm")
            nc.vector.tensor_reduce(out=n_[:], in_=sq[:],
                                    axis=mybir.AxisListType.X,
                                    op=mybir.AluOpType.add)
            nc.scalar.activation(out=n_[:], in_=n_[:],
                                 func=mybir.ActivationFunctionType.Sqrt)
            nc.vector.tensor_scalar_max(out=n_[:], in0=n_[:], scalar1=EPS_LOG)
            a_ = spool.tile([P, CH], F32, tag="a_", name="a_")
            nc.vector.tensor_scalar(out=a_[:], in0=n_[:],
                                    scalar1=sc_t[:, 0:1], scalar2=None,
                                    op0=mybir.AluOpType.mult)
            t1 = spool.tile([P, CH], F32, tag="t1", name="t1")
            nc.scalar.activation(out=t1[:], in_=a_[:],
                                 func=mybir.ActivationFunctionType.Ln,
                                 bias=1.0, scale=1.0)
            t2 = spool.tile([P, CH], F32, tag="t2", name="t2")
            nc.scalar.activation(out=t2[:], in_=a_[:],
                                 func=mybir.ActivationFunctionType.Ln,
                                 bias=1.0, scale=-1.0)
            f_ = spool.tile([P, CH], F32, tag="f_", name="f_")
            nc.vector.tensor_tensor(out=f_[:], in0=t1[:], in1=t2[:],
                                    op=mybir.AluOpType.subtract)
            rn = spool.tile([P, CH], F32, tag="rn", name="rn")
            nc.vector.reciprocal(out=rn[:], in_=n_[:])
            nc.vector.tensor_tensor(out=f_[:], in0=f_[:], in1=rn[:],
                                    op=mybir.AluOpType.mult)
            nc.vector.tensor_scalar(out=f_[:], in0=f_[:],
                                    scalar1=inv_sc_t[:, 0:1], scalar2=None,
                                    op0=mybir.AluOpType.mult)
            tan = spool.tile([P, CH, D], F32, tag="tan", name="tan")
            for j in range(CH):
                nc.vector.tensor_scalar_mul(out=tan[:, j, :],
                                            in0=bpart[:, j, :],
                                            scalar1=f_[:, j : j + 1])

            sn = spool.tile([P, CH, D], F32, tag="sn", name="sn")
            nc.vector.tensor_copy(out=sn[:], in_=ch[:, :, 2 * D : 3 * D])
            l2norm_chunk(sn[:], CH, EPS_L2)

            xch = spool.tile([P, CH, DX], XDT, tag="xch", name="xch")
            for j in range(CH):
                ins_nm = (ch[:, j, 0:D], tan[:, j, :], sn[:, j, :])
                for sp_ in range(3):
                    tp = ppool2.tile([P, P], F32, space="PSUM", tag="tp",
                                     name="tp")
                    nc.tensor.transpose(out=tp[:], in_=ins_nm[sp_],
                                        identity=ident_t[:])
                    tsb = spool.tile([P, P], BF16, tag="tsb", name="tsb")
                    nc.any.tensor_copy(out=tsb[:], in_=tp[:])
                    xp = ppool2.tile([P, D], F32, space="PSUM", tag="xp",
                                     name="xp")
                    nc.tensor.matmul(xp[:], lhsT=tsb[:], rhs=wT_t[l][sp_][:],
                                     start=True, stop=True)
                    # bias add fused into the PSUM -> SBUF copy
                    nc.vector.tensor_tensor(
                        out=xch[:, j, sp_ * D : (sp_ + 1) * D],
                        in0=xp[:], in1=bias_t[l][sp_][:],
                        op=mybir.AluOpType.add)
            l2norm_chunk(xch[:, :, 2 * D : 3 * D], CH, EPS_L2)
            if g < SPC:
                nc.sync.dma_start(out=rows3d(x_locA[l][:], r0, CH), in_=xch[:])
            else:
                nc.sync.dma_start(
                    out=rows3d(x_locB[l][:], r0 - RA, CH), in_=xch[:])

        def emit_allgather_a(l):
            nc.gpsimd.collective_compute(
                "AllGather", mybir.AluOpType.bypass,
                replica_groups=[list(range(NC_))],
                ins=[x_locA[l][:]], outs=[xfA[l][:]])

        def emit_allgather_b(l):
            nc.gpsimd.collective_compute(
                "AllGather", mybir.AluOpType.bypass,
                replica_groups=[list(range(NC_))],
                ins=[x_locB[l][:]], outs=[xfB[l][:]])

        gcnt_regs = [nc.gpsimd.alloc_register(f"gcnt_reg{i}")
                     for i in range(8)]
        _gri = [0]

        def gather_ordinal(w, half, off):
            # position of this sub-gather in host_prep's gcnt emission order
            gi = 0
            for w2 in range(w):
                gi += (K_lo[w2] + 1023) // 1024 + (K_hi[w2] + 1023) // 1024
            if half == 1:
                gi += (K_lo[w] + 1023) // 1024
            return gi + off // 1024

        def emit_agg_group(l, grp):
            pb = spool.tile([P, CH, DX], F32, tag="postbuf", name="postbuf")
            for wi in range(CH):
                w = grp * CH + wi
                tb = tile_base[w]
                gb = gpool.tile([P, Tmax, DX], XDT, tag="gb", name="gb")
                GMAX = 1024  # max idxs per dma_gather (descriptor ring cap)
                for half, K, t0_ in ((0, K_lo[w], 0), (1, K_hi[w], T_lo[w])):
                    src_ap = xfA[l][:] if half == 0 else xfB[l][:]
                    for off in range(0, K, GMAX):
                        kk = min(GMAX, K - off)
                        tstart = t0_ + off // P
                        cb = (tb + tstart) * P // 16
                        gi = gather_ordinal(w, half, off)
                        rv = gcnt_regs[_gri[0] % 8]
                        # rotate desc-gen across the 4 SWDGE queues: each
                        # queue runs on its own Q7 core pair, so gathers on
                        # different queues generate descriptors in parallel
                        qn = _gri[0] % 4
                        _gri[0] += 1
                        nc.gpsimd.reg_load(rv, gcnt_t[0:1, gi : gi + 1])
                        nc.gpsimd.dma_gather(
                            out_ap=gb[:, tstart : tstart + kk // P, :],
                            in_ap=src_ap,
                            idxs_ap=idx_t[:, cb : cb + kk // 16],
                            num_idxs=kk, num_idxs_reg=rv,
                            elem_size=DX, queue_num=qn)
                acc = ppool.tile([P, DX], F32, space="PSUM", tag="acc",
                                 name="acc")
                for t in range(T[w]):
                    r_ = rpool.tile([P, P], XDT, tag="r", name="r")
                    nc.vector.tensor_tensor(
                        out=r_[:],
                        in0=dstv_bf[:, tb + t : tb + t + 1].to_broadcast([P, P]),
                        in1=iota_bf[:],
                        op=mybir.AluOpType.is_equal)
                    nc.tensor.matmul(acc[:], lhsT=r_[:], rhs=gb[:, t, :],
                                     start=(t == 0), stop=(t == T[w] - 1))
                nc.vector.tensor_scalar_mul(out=pb[:, wi, :], in0=acc[:],
                                            scalar1=recip_t[:, w : w + 1])

            # post pointwise
            epart = pb[:, :, 0:D]
            tmp = spool.tile([P, CH, D], F32, tag="psq", name="psq")
            nc.vector.tensor_scalar_mul(out=tmp[:], in0=epart, scalar1=0.2)
            nc.vector.tensor_tensor(out=epart, in0=epart, in1=tmp[:],
                                    op=mybir.AluOpType.max)
            bpart = pb[:, :, D : 2 * D]
            nc.vector.tensor_tensor(out=tmp[:], in0=bpart, in1=bpart,
                                    op=mybir.AluOpType.mult)
            n_ = spool.tile([P, CH], F32, tag="pnrm", name="pnrm")
            nc.vector.tensor_reduce(out=n_[:], in_=tmp[:],
                                    axis=mybir.AxisListType.X,
                                    op=mybir.AluOpType.add)
            nc.scalar.activation(out=n_[:], in_=n_[:],
                                 func=mybir.ActivationFunctionType.Sqrt)
            nc.vector.tensor_scalar_max(out=n_[:], in0=n_[:], scalar1=EPS_LOG)
            a_ = spool.tile([P, CH], F32, tag="pa_", name="pa_")
            nc.vector.tensor_scalar(out=a_[:], in0=n_[:],
                                    scalar1=sc_t[:, 0:1], scalar2=None,
                                    op0=mybir.AluOpType.mult)
            th = spool.tile([P, CH], F32, tag="pt1", name="pt1")
            nc.scalar.activation(out=th[:], in_=a_[:],
                                 func=mybir.ActivationFunctionType.Tanh,
                                 scale=0.5)
            ra = spool.tile([P, CH], F32, tag="pt2", name="pt2")
            nc.vector.reciprocal(out=ra[:], in_=a_[:])
            nc.vector.tensor_tensor(out=th[:], in0=th[:], in1=ra[:],
                                    op=mybir.AluOpType.mult)
            for j in range(CH):
                nc.vector.tensor_scalar_mul(out=bpart[:, j, :],
                                            in0=bpart[:, j, :],
                                            scalar1=th[:, j : j + 1])
            l2norm_chunk(pb[:, :, 2 * D : 3 * D], CH, EPS_L2, pfx="p")

            if l == L - 1:
                nc.sync.dma_start(
                    out=rows3d(out_d[:], grp * CH * P, CH), in_=pb[:])
            else:
                nc.sync.dma_start(
                    out=rows3d(emb_mid[grp][:], 0, CH), in_=pb[:])

        # driver: interleave layer-(l+1) build chunks into layer-l agg
        # groups; AllGather half A fires once its chunks are built (half
        # tensors keep the WAR deps exact).
        for g in range(NG):
            emit_build_chunk(0, g)
            if g == SPC - 1:
                emit_allgather_a(0)
        emit_allgather_b(0)
        for g in range(NG):
            emit_agg_group(0, g)
            emit_build_chunk(1, g)
            if g == SPC - 1:
                emit_allgather_a(1)
        emit_allgather_b(1)
        for g in range(NG):
            emit_agg_group(1, g)

    return nc


def _build_in_maps(cfg, src, dst, e_emb, b_emb, s_emb, e_W, e_b, b_W, b_b,
                   s_W, s_b, b_curvature):
    N, SH, NC, DX, L, D = cfg.N, cfg.SH, cfg.NC, cfg.DX, cfg.L, cfg.D
    idx_all, dstv, deg_arr, gcnt, meta = _host_prep(cfg, src, dst)

    emb_full = np.zeros((cfg.NPAD, DX), np.float32)
    emb_full[:N, 0:D] = e_emb
    emb_full[:N, D:2 * D] = b_emb
    emb_full[:N, 2 * D:3 * D] = s_emb

    wT = np.stack([
        np.stack([e_W[l].T, b_W[l].T, s_W[l].T]) for l in range(L)
    ]).astype(np.float32)
    bias = np.stack([
        np.stack([e_b[l], b_b[l], s_b[l]]) for l in range(L)
    ]).astype(np.float32)
    # broadcast per-feature bias across the 128 node partitions
    bias_bc = np.ascontiguousarray(
        np.broadcast_to(bias[:, :, None, :], (L, 3, P, D)), dtype=np.float32)

    iota = np.tile(np.arange(P, dtype=np.float32), (P, 1))
    curv = np.full((P, 1), np.float32(np.asarray(b_curvature).reshape(-1)[0]))

    in_maps = []
    for c in range(NC):
        in_maps.append({
            "emb0": np.ascontiguousarray(emb_full[c * SH:(c + 1) * SH]),
            "wT": wT,
            "bias": bias_bc,
            "idx": np.ascontiguousarray(idx_all[c]),
            "dstv": np.ascontiguousarray(dstv[c]),
            "deg": np.ascontiguousarray(deg_arr[c]),
            "iota": iota,
            "curv": curv,
            "gcnt": np.ascontiguousarray(gcnt[c]),
        })
    return in_maps, meta


_LAST = {}


def run_kernel(inputs, trace=False):
    """Full pipeline; returns (results, exec_time_ns)."""
    from concourse.bass_utils import run_bass_kernel_spmd

    cfg = _Cfg(n_nodes=50000, n_edges=800000)
    src = np.asarray(inputs["src"], np.int32)
    dst = np.asarray(inputs["dst"], np.int32)
    in_maps, meta = _build_in_maps(
        cfg, src, dst,
        np.asarray(inputs["e_emb"], np.float32),
        np.asarray(inputs["b_emb"], np.float32),
        np.asarray(inputs["s_emb"], np.float32),
        np.asarray(inputs["e_W"], np.float32),
        np.asarray(inputs["e_b"], np.float32),
        np.asarray(inputs["b_W"], np.float32),
        np.asarray(inputs["b_b"], np.float32),
        np.asarray(inputs["s_W"], np.float32),
        np.asarray(inputs["s_b"], np.float32),
        np.asarray(inputs["b_curvature"], np.float32))

    # reuse the compiled program across calls when the graph layout matches
    gkey = (meta["TT"], hash(src.tobytes()) ^ hash(dst.tobytes()))
    nc = _LAST.get(gkey)
    if nc is None:
        nc = _build_nc(cfg, meta)
        nc.finalize()
        _split_excess_waits(nc)
        _LAST.clear()
        _LAST[gkey] = nc

    res = run_bass_kernel_spmd(nc, in_maps, core_ids=list(range(cfg.NC)),
                               trace=trace)
    full = np.concatenate([res.results[c]["out"] for c in range(cfg.NC)],
                          axis=0)[: cfg.N]
    D = cfg.D
    outs = (np.ascontiguousarray(full[:, 0:D]),
            np.ascontiguousarray(full[:, D:2 * D]),
            np.ascontiguousarray(full[:, 2 * D:3 * D]))
    return outs, res.exec_time_ns


def kernel(**inputs):
    outs, _ = run_kernel(inputs, trace=False)
    return outs

